# revision 13
# baseline (speedup 1.0000x reference)
"""DKT-PEBG kernel for Trainium2 (8 NeuronCores, batch-parallel).

Model: embedding lookup -> masked concat -> LSTM(128) -> per-token output
probability via gathered W_out rows (avoids materializing [B,S,10000]).

Sharding: data-parallel over batch. Core c handles batch rows [8c, 8c+8).
No collectives; host splits inputs / concatenates outputs.

Shapes (hardcoded): B=64, S=200, E=H=128, PRO_NUM=10000.

Recurrence trick: gate order [i,f,o,g] with the g-gate pre-activation
prescaled by 2 on the host, so one Sigmoid over all 4 gates suffices:
tanh(g) = 2*sigmoid(2g) - 1. Input-GEMM chunks and gathers are emitted
interleaved with the first recurrence steps so the scheduler pipelines
them instead of serializing ~45us of startup.
"""

import numpy as np

import concourse.bass as bass
import concourse.bacc as bacc
import concourse.mybir as mybir
import concourse.tile as tile
from concourse.bass_utils import run_bass_kernel_spmd
from concourse.masks import make_identity

B, S = 64, 200
E = 128
H = 128
PRO_NUM = 10000
N_CORES = 8
BS = B // N_CORES              # 8 batch rows per core
NT = BS * S                    # 1600 tokens per core, token n = 8*s + b
NTILES = 13                    # ceil(1600/128); tile 12 has 64 valid tokens
NOUT = BS * (S - 1)            # 1592 output tokens
WB = H + 1                     # gathered W_out row + bias
F32 = mybir.dt.float32
I32 = mybir.dt.int32

_GATE_SRC = (0, 1, 3, 2)       # col blocks [i, f, o, g] <- W_ih row blocks (i,f,g,o)

# input-GEMM chunks in tiles: (first_tile, n_tiles); chunk 0 small so the
# recurrence can start early
_CHUNKS = ((0, 1), (1, 4), (5, 4), (9, 4))


def _tok_w(t):
    return 128 if t < NTILES - 1 else NT - 128 * (NTILES - 1)


def _out_w(t):
    return 128 if t < NTILES - 1 else NOUT - 128 * (NTILES - 1)


def build_kernel():
    nc = bacc.Bacc("TRN2", target_bir_lowering=False, debug=False,
                   num_devices=N_CORES)

    # ---- I/O ----
    xt = nc.dram_tensor("xt", [209, BS], I32, kind="ExternalInput")   # X.T slice, padded
    yt = nc.dram_tensor("yt", [208, BS], I32, kind="ExternalInput")   # y.T slice, padded
    emb = nc.dram_tensor("emb", [PRO_NUM, E], F32, kind="ExternalInput")
    wx = nc.dram_tensor("wx", [128, 1024], F32, kind="ExternalInput")  # W_ih.T blocks [A|B]
    whh = nc.dram_tensor("whh", [128, 512], F32, kind="ExternalInput")  # W_hh.T blocks
    bsum = nc.dram_tensor("bsum", [128, 4], F32, kind="ExternalInput")  # b_ih+b_hh blocks
    wb = nc.dram_tensor("wb", [PRO_NUM, WB], F32, kind="ExternalInput")  # [W_out | b_out]
    prob = nc.dram_tensor("prob", [NTILES * 128], F32, kind="ExternalOutput")

    AF = mybir.ActivationFunctionType
    OP = mybir.AluOpType

    with tile.TileContext(nc) as tc:
        with (
            tc.tile_pool(name="persist", bufs=1) as pp,
            tc.tile_pool(name="work", bufs=3) as wp,
            tc.tile_pool(name="rec", bufs=3) as rp,
            tc.tile_pool(name="ps_tr", bufs=2, space="PSUM") as ps_tr,
            tc.tile_pool(name="ps_mm", bufs=3, space="PSUM") as ps_mm,
            tc.tile_pool(name="ps_rec", bufs=3, space="PSUM") as ps_rec,
        ):
            # ---- persistent SBUF ----
            ident = pp.tile([128, 128], F32, tag="ident")
            wx_sb = pp.tile([128, 1024], F32, tag="wx_sb")
            whh_sb = pp.tile([128, 512], F32, tag="whh_sb")
            bias_sb = pp.tile([128, 4], F32, tag="bias_sb")
            ix_all = pp.tile([128, NTILES], I32, tag="ix_all")
            ixs_all = pp.tile([128, NTILES], I32, tag="ixs_all")
            y_all = pp.tile([128, NTILES], I32, tag="y_all")
            y_f = pp.tile([128, NTILES], F32, tag="y_f")
            m1 = pp.tile([128, NTILES], F32, tag="m1")
            m2 = pp.tile([128, NTILES], F32, tag="m2")
            ixm1 = pp.tile([128, NTILES], I32, tag="ixm1")
            ixs_f = pp.tile([128, NTILES], F32, tag="ixs_f")
            mnz = pp.tile([128, NTILES], F32, tag="mnz")
            xaT = pp.tile([128, NT], F32, tag="xaT")
            xbT = pp.tile([128, NT], F32, tag="xbT")
            xgb = pp.tile([128, 32 * S], F32, tag="xgb")
            hseq = pp.tile([128, NT], F32, tag="hseq")
            wgb_all = pp.tile([128, NTILES * WB], F32, tag="wgb_all")
            c_st = pp.tile([128, BS], F32, tag="c_st")
            prob_sb = pp.tile([128, NTILES], F32, tag="prob_sb")

            make_identity(nc, ident[:])
            nc.gpsimd.memset(prob_sb[:], 0.0)

            # ---- loads ----
            xt_flat = xt[:].rearrange("s b -> (s b)")
            yt_flat = yt[:].rearrange("s b -> (s b)")
            nc.sync.dma_start(
                ix_all[:], xt_flat[0:1664].rearrange("(t p) -> p t", p=128))
            nc.sync.dma_start(
                y_all[:], yt_flat[0:1664].rearrange("(t p) -> p t", p=128))
            nc.sync.dma_start(
                ixs_all[:], xt_flat[8:1672].rearrange("(t p) -> p t", p=128))
            nc.sync.dma_start(wx_sb[:], wx[:])
            nc.sync.dma_start(whh_sb[:], whh[:])
            nc.sync.dma_start(bias_sb[:], bsum[:])

            # masks: m1 = (y==0), m2 = (y==1); padding y==-1 -> 0,0
            nc.vector.tensor_copy(y_f[:], y_all[:])
            nc.vector.tensor_scalar(m1[:], y_f[:], 0.0, None, op0=OP.is_equal)
            nc.vector.tensor_scalar(m2[:], y_f[:], 1.0, None, op0=OP.is_equal)
            # ixm1 = max(X[s+1]-1, 0), mnz = (X[s+1] != 0)
            nc.vector.tensor_scalar(ixm1[:], ixs_all[:], 1, 0,
                                    op0=OP.subtract, op1=OP.max)
            nc.vector.tensor_copy(ixs_f[:], ixs_all[:])
            nc.vector.tensor_scalar(mnz[:], ixs_f[:], 0.0, None,
                                    op0=OP.not_equal)

            def process_tile(t):
                """gather embeddings for tile t, mask, transpose into xaT/xbT"""
                w = _tok_w(t)
                ex_t = wp.tile([128, E], F32, tag="ex")
                nc.gpsimd.indirect_dma_start(
                    out=ex_t[0:w, :], out_offset=None, in_=emb[:],
                    in_offset=bass.IndirectOffsetOnAxis(
                        ap=ix_all[0:w, t:t + 1], axis=0))
                xa_t = wp.tile([128, E], F32, tag="xa")
                xb_t = wp.tile([128, E], F32, tag="xb")
                nc.vector.tensor_scalar(xa_t[0:w, :], ex_t[0:w, :],
                                        m1[0:w, t:t + 1], None, op0=OP.mult)
                nc.vector.tensor_scalar(xb_t[0:w, :], ex_t[0:w, :],
                                        m2[0:w, t:t + 1], None, op0=OP.mult)
                psa = ps_tr.tile([128, 128], F32, tag="psa")
                nc.tensor.transpose(psa[:, 0:w], xa_t[0:w, :], ident[0:w, 0:w])
                nc.vector.tensor_copy(xaT[:, 128 * t:128 * t + w], psa[:, 0:w])
                psb = ps_tr.tile([128, 128], F32, tag="psa")
                nc.tensor.transpose(psb[:, 0:w], xb_t[0:w, :], ident[0:w, 0:w])
                nc.vector.tensor_copy(xbT[:, 128 * t:128 * t + w], psb[:, 0:w])

            def gemm_pair(c, j):
                """one gate's input GEMM for chunk c + evac into xgb layout"""
                t0, ntl = _CHUNKS[c]
                base = 128 * t0
                w = min(128 * ntl, NT - base)
                psg = ps_mm.tile([128, 512], F32, tag="psg")
                nc.tensor.matmul(
                    psg[:, 0:w], wx_sb[:, 128 * j:128 * (j + 1)],
                    xaT[:, base:base + w], start=True, stop=False)
                nc.tensor.matmul(
                    psg[:, 0:w], wx_sb[:, 512 + 128 * j:512 + 128 * (j + 1)],
                    xbT[:, base:base + w], start=False, stop=True)
                dst = xgb[:, 4 * base: 4 * base + 32 * (w // 8)] \
                    .rearrange("p (q x) -> p q x", x=32)[:, :, 8 * j:8 * j + 8]
                src = psg[:, 0:w].rearrange("p (q x) -> p q x", x=8)
                nc.vector.tensor_scalar(dst, src, bias_sb[:, j:j + 1], None,
                                        op0=OP.add)

            def gather_wb(t):
                w = _out_w(t)
                nc.gpsimd.indirect_dma_start(
                    out=wgb_all[0:w, WB * t:WB * (t + 1)], out_offset=None,
                    in_=wb[:],
                    in_offset=bass.IndirectOffsetOnAxis(
                        ap=ixm1[0:w, t:t + 1], axis=0))

            def out_tile(t):
                '''prob = sigmoid(h . W_out[idx] + b_out[idx]) * (X != 0)'''
                w = _out_w(t)
                pst = ps_tr.tile([128, 128], F32, tag="psa")
                nc.tensor.transpose(pst[0:w, :], hseq[:, 128 * t:128 * t + w],
                                    ident[:])
                hw_t = wp.tile([128, 128], F32, tag="hw")
                d_t = wp.tile([128, 1], F32, tag="d")
                nc.vector.tensor_tensor(out=hw_t[0:w, :], in0=pst[0:w, :],
                                        in1=wgb_all[0:w, WB * t:WB * t + H],
                                        op=OP.mult)
                nc.vector.tensor_reduce(d_t[0:w, :], hw_t[0:w, :],
                                        axis=mybir.AxisListType.X, op=OP.add)
                p_t = wp.tile([128, 1], F32, tag="p")
                nc.scalar.activation(p_t[0:w, :], d_t[0:w, :], AF.Sigmoid,
                                     bias=wgb_all[0:w, WB * t + H:WB * (t + 1)])
                nc.vector.tensor_tensor(out=prob_sb[0:w, t:t + 1],
                                        in0=p_t[0:w, :],
                                        in1=mnz[0:w, t:t + 1], op=OP.mult)

            # interleave schedule: step index -> list of thunks.
            # chunk c tokens start at step 16*_CHUNKS[c][0]; stay well ahead.
            side = {}
            for c in (1, 2, 3):
                base = 16 * (c - 1)
                for k in range(4):              # tiles of chunk c
                    side.setdefault(base + 2 * k + 1, []).append(
                        ("tile", _CHUNKS[c][0] + k))
                for j in range(4):              # gemm pairs of chunk c
                    side.setdefault(base + 2 * j + 9, []).append(("gemm", c, j))
            for t in range(NTILES):             # wgb gathers
                side.setdefault(49 + 2 * t, []).append(("wb", t))
            late_out = []
            for t in range(NTILES):             # output tiles once h is ready
                step = max(16 * t + 17, 80 + t)
                if step <= S - 1:
                    side.setdefault(step, []).append(("out", t))
                else:
                    late_out.append(t)

            # ---- chunk 0 then the recurrence with interleaved side work ----
            process_tile(0)
            for j in range(4):
                gemm_pair(0, j)

            for t in range(S):
                psr = ps_rec.tile([128, 32], F32, tag="psr")
                nc.tensor.matmul(psr[:], ident[:], xgb[:, 32 * t:32 * t + 32],
                                 start=True, stop=(t == 0))
                if t > 0:
                    hprev = hseq[:, 8 * (t - 1):8 * t]
                    for j in range(4):
                        nc.tensor.matmul(
                            psr[:, 8 * j:8 * j + 8],
                            whh_sb[:, 128 * j:128 * (j + 1)], hprev,
                            start=False, stop=(j == 3))
                # cols [i|f|o|g]; g was prescaled x2 => tanh(g) = 2*sig-1
                sig = rp.tile([128, 32], F32, tag="sig")
                nc.scalar.activation(sig[:], psr[:], AF.Sigmoid)
                # u = si*(2*sg-1) = 2*w, w = (sg-0.5)*si ; c = f*c + 2w
                w_t = rp.tile([128, 8], F32, tag="w_t")
                nc.vector.scalar_tensor_tensor(
                    out=w_t[:], in0=sig[:, 24:32], scalar=0.5,
                    in1=sig[:, 0:8], op0=OP.subtract, op1=OP.mult)
                if t == 0:
                    nc.vector.tensor_scalar(c_st[:], w_t[:], 2.0, None,
                                            op0=OP.mult)
                else:
                    cf = rp.tile([128, 8], F32, tag="cf")
                    nc.vector.tensor_tensor(out=cf[:], in0=sig[:, 8:16],
                                            in1=c_st[:], op=OP.mult)
                    nc.vector.scalar_tensor_tensor(
                        out=c_st[:], in0=w_t[:], scalar=2.0, in1=cf[:],
                        op0=OP.mult, op1=OP.add)
                tch = rp.tile([128, 8], F32, tag="tch")
                nc.scalar.activation(tch[:], c_st[:], AF.Tanh)
                nc.vector.tensor_tensor(out=hseq[:, 8 * t:8 * t + 8],
                                        in0=sig[:, 16:24], in1=tch[:], op=OP.mult)

                for item in side.get(t, ()):
                    if item[0] == "tile":
                        process_tile(item[1])
                    elif item[0] == "gemm":
                        gemm_pair(item[1], item[2])
                    elif item[0] == "wb":
                        gather_wb(item[1])
                    else:
                        out_tile(item[1])

            for t in late_out:
                out_tile(t)

            nc.sync.dma_start(
                prob[:].rearrange("(t p) -> p t", p=128), prob_sb[:])

    nc.compile()
    return nc


_CACHED = None


def _get_kernel():
    global _CACHED
    if _CACHED is None:
        _CACHED = build_kernel()
    return _CACHED


def _prep_shared(pro_embed, W_ih, W_hh, b_ih, b_hh, W_out, b_out):
    wx_h = np.empty((128, 1024), np.float32)
    whh_h = np.empty((128, 512), np.float32)
    bias_h = np.empty((128, 4), np.float32)
    for j, g in enumerate(_GATE_SRC):
        blk = slice(g * 128, (g + 1) * 128)
        sc = 2.0 if j == 3 else 1.0   # g-gate prescale: tanh(x)=2*sig(2x)-1
        wx_h[:, j * 128:(j + 1) * 128] = sc * W_ih[blk, 0:128].T
        wx_h[:, 512 + j * 128:512 + (j + 1) * 128] = sc * W_ih[blk, 128:256].T
        whh_h[:, j * 128:(j + 1) * 128] = sc * W_hh[blk, :].T
        bias_h[:, j] = sc * (b_ih[blk] + b_hh[blk])
    wb_h = np.empty((PRO_NUM, WB), np.float32)
    wb_h[:, :H] = W_out
    wb_h[:, H] = b_out
    return dict(
        emb=np.ascontiguousarray(pro_embed, np.float32),
        wx=np.ascontiguousarray(wx_h),
        whh=np.ascontiguousarray(whh_h),
        bsum=np.ascontiguousarray(bias_h),
        wb=wb_h,
    )


def kernel(X, y, pro_embed, W_ih, W_hh, b_ih, b_hh, W_out, b_out, _trace=False,
           **_):
    X = np.asarray(X, np.int32)
    y = np.asarray(y, np.int32)
    shared = _prep_shared(np.asarray(pro_embed, np.float32),
                          np.asarray(W_ih, np.float32),
                          np.asarray(W_hh, np.float32),
                          np.asarray(b_ih, np.float32),
                          np.asarray(b_hh, np.float32),
                          np.asarray(W_out, np.float32),
                          np.asarray(b_out, np.float32))
    XT = X.T  # [200, 64]
    YT = y.T
    in_maps = []
    for c in range(N_CORES):
        cols = slice(c * BS, (c + 1) * BS)
        xtp = np.zeros((209, BS), np.int32)
        xtp[:S] = XT[:, cols]
        ytp = np.zeros((208, BS), np.int32)
        ytp[:S] = YT[:, cols]
        in_maps.append(dict(xt=xtp, yt=ytp, **shared))

    nc = _get_kernel()
    res = run_bass_kernel_spmd(nc, in_maps, core_ids=list(range(N_CORES)),
                               trace=_trace)
    out = np.empty((B, S - 1), np.float32)
    for c in range(N_CORES):
        flat = res.results[c]["prob"][:NOUT].reshape(S - 1, BS)
        out[c * BS:(c + 1) * BS, :] = flat.T
    if _trace:
        return out, res
    return out


# revision 15
# speedup vs baseline: 253.8817x; 253.8817x over previous
"""DKT-PEBG kernel for Trainium2 (8 NeuronCores, batch-parallel).

Model: embedding lookup -> masked concat -> LSTM(128) -> per-token output
probability via gathered W_out rows (avoids materializing [B,S,10000]).

Sharding: data-parallel over batch. Core c handles batch rows [8c, 8c+8).
No collectives; host splits inputs / concatenates outputs.

Shapes (hardcoded): B=64, S=200, E=H=128, PRO_NUM=10000.

Recurrence trick: gate order [i,f,o,g] with the g-gate pre-activation
prescaled by 2 on the host, so one Sigmoid over all 4 gates suffices:
tanh(g) = 2*sigmoid(2g) - 1. Input-GEMM chunks and gathers are emitted
interleaved with the first recurrence steps so the scheduler pipelines
them instead of serializing ~45us of startup.
"""

import numpy as np

import concourse.bass as bass
import concourse.bacc as bacc
import concourse.mybir as mybir
import concourse.tile as tile
from concourse.bass_utils import run_bass_kernel_spmd
from concourse.masks import make_identity

B, S = 64, 200
E = 128
H = 128
PRO_NUM = 10000
N_CORES = 8
BS = B // N_CORES              # 8 batch rows per core
NT = BS * S                    # 1600 tokens per core, token n = 8*s + b
NTILES = 13                    # ceil(1600/128); tile 12 has 64 valid tokens
NOUT = BS * (S - 1)            # 1592 output tokens
WB = H + 1                     # gathered W_out row + bias
F32 = mybir.dt.float32
I32 = mybir.dt.int32

_GATE_SRC = (0, 1, 3, 2)       # col blocks [i, f, o, g] <- W_ih row blocks (i,f,g,o)

# input-GEMM chunks in tiles: (first_tile, n_tiles); chunk 0 small so the
# recurrence can start early
_CHUNKS = ((0, 1), (1, 4), (5, 4), (9, 4))


def _tok_w(t):
    return 128 if t < NTILES - 1 else NT - 128 * (NTILES - 1)


def _out_w(t):
    return 128 if t < NTILES - 1 else NOUT - 128 * (NTILES - 1)


def build_kernel():
    nc = bacc.Bacc("TRN2", target_bir_lowering=False, debug=False,
                   num_devices=N_CORES)

    # ---- I/O ----
    xt = nc.dram_tensor("xt", [209, BS], I32, kind="ExternalInput")   # X.T slice, padded
    yt = nc.dram_tensor("yt", [208, BS], I32, kind="ExternalInput")   # y.T slice, padded
    emb = nc.dram_tensor("emb", [PRO_NUM, E], F32, kind="ExternalInput")
    wx = nc.dram_tensor("wx", [128, 1024], F32, kind="ExternalInput")  # W_ih.T blocks [A|B]
    whh = nc.dram_tensor("whh", [128, 512], F32, kind="ExternalInput")  # W_hh.T blocks
    bsum = nc.dram_tensor("bsum", [128, 4], F32, kind="ExternalInput")  # b_ih+b_hh blocks
    wb = nc.dram_tensor("wb", [PRO_NUM, WB], F32, kind="ExternalInput")  # [W_out | b_out]
    prob = nc.dram_tensor("prob", [NTILES * 128], F32, kind="ExternalOutput")

    AF = mybir.ActivationFunctionType
    OP = mybir.AluOpType

    with tile.TileContext(nc) as tc:
        with (
            tc.tile_pool(name="persist", bufs=1) as pp,
            tc.tile_pool(name="work", bufs=3) as wp,
            tc.tile_pool(name="rec", bufs=3) as rp,
            tc.tile_pool(name="ps_tr", bufs=2, space="PSUM") as ps_tr,
            tc.tile_pool(name="ps_mm", bufs=3, space="PSUM") as ps_mm,
            tc.tile_pool(name="ps_rec", bufs=3, space="PSUM") as ps_rec,
        ):
            # ---- persistent SBUF ----
            ident = pp.tile([128, 128], F32, tag="ident")
            wx_sb = pp.tile([128, 1024], F32, tag="wx_sb")
            whh_sb = pp.tile([128, 512], F32, tag="whh_sb")
            bias_sb = pp.tile([128, 4], F32, tag="bias_sb")
            ix_all = pp.tile([128, NTILES], I32, tag="ix_all")
            ixs_all = pp.tile([128, NTILES], I32, tag="ixs_all")
            y_all = pp.tile([128, NTILES], I32, tag="y_all")
            y_f = pp.tile([128, NTILES], F32, tag="y_f")
            m1 = pp.tile([128, NTILES], F32, tag="m1")
            m2 = pp.tile([128, NTILES], F32, tag="m2")
            ixm1 = pp.tile([128, NTILES], I32, tag="ixm1")
            ixs_f = pp.tile([128, NTILES], F32, tag="ixs_f")
            mnz = pp.tile([128, NTILES], F32, tag="mnz")
            xaT = pp.tile([128, NT], F32, tag="xaT")
            xbT = pp.tile([128, NT], F32, tag="xbT")
            xgb = pp.tile([128, 32 * S], F32, tag="xgb")
            hseq = pp.tile([128, NT], F32, tag="hseq")
            wgb_all = pp.tile([128, NTILES * WB], F32, tag="wgb_all")
            c_st = pp.tile([128, BS], F32, tag="c_st")
            prob_sb = pp.tile([128, NTILES], F32, tag="prob_sb")

            make_identity(nc, ident[:])
            nc.gpsimd.memset(prob_sb[:], 0.0)

            # ---- loads ----
            xt_flat = xt[:].rearrange("s b -> (s b)")
            yt_flat = yt[:].rearrange("s b -> (s b)")
            nc.sync.dma_start(
                ix_all[:], xt_flat[0:1664].rearrange("(t p) -> p t", p=128))
            nc.sync.dma_start(
                y_all[:], yt_flat[0:1664].rearrange("(t p) -> p t", p=128))
            nc.sync.dma_start(
                ixs_all[:], xt_flat[8:1672].rearrange("(t p) -> p t", p=128))
            nc.sync.dma_start(wx_sb[:], wx[:])
            nc.sync.dma_start(whh_sb[:], whh[:])
            nc.sync.dma_start(bias_sb[:], bsum[:])

            # warm the ACT sigmoid/tanh table set off the critical path
            warm = wp.tile([1, 1], F32, tag="warm")
            nc.scalar.activation(warm[:], ident[0:1, 0:1], AF.Sigmoid)

            # masks: m1 = (y==0), m2 = (y==1); padding y==-1 -> 0,0
            nc.vector.tensor_copy(y_f[:], y_all[:])
            nc.vector.tensor_scalar(m1[:], y_f[:], 0.0, None, op0=OP.is_equal)
            nc.vector.tensor_scalar(m2[:], y_f[:], 1.0, None, op0=OP.is_equal)
            # ixm1 = max(X[s+1]-1, 0), mnz = (X[s+1] != 0)
            nc.vector.tensor_scalar(ixm1[:], ixs_all[:], 1, 0,
                                    op0=OP.subtract, op1=OP.max)
            nc.vector.tensor_copy(ixs_f[:], ixs_all[:])
            nc.vector.tensor_scalar(mnz[:], ixs_f[:], 0.0, None,
                                    op0=OP.not_equal)

            def process_tile(t):
                """gather embeddings for tile t, mask, transpose into xaT/xbT"""
                w = _tok_w(t)
                ex_t = wp.tile([128, E], F32, tag="ex")
                nc.gpsimd.indirect_dma_start(
                    out=ex_t[0:w, :], out_offset=None, in_=emb[:],
                    in_offset=bass.IndirectOffsetOnAxis(
                        ap=ix_all[0:w, t:t + 1], axis=0))
                xa_t = wp.tile([128, E], F32, tag="xa")
                xb_t = wp.tile([128, E], F32, tag="xb")
                nc.vector.tensor_scalar(xa_t[0:w, :], ex_t[0:w, :],
                                        m1[0:w, t:t + 1], None, op0=OP.mult)
                nc.vector.tensor_scalar(xb_t[0:w, :], ex_t[0:w, :],
                                        m2[0:w, t:t + 1], None, op0=OP.mult)
                psa = ps_tr.tile([128, 128], F32, tag="psa")
                nc.tensor.transpose(psa[:, 0:w], xa_t[0:w, :], ident[0:w, 0:w])
                nc.vector.tensor_copy(xaT[:, 128 * t:128 * t + w], psa[:, 0:w])
                psb = ps_tr.tile([128, 128], F32, tag="psa")
                nc.tensor.transpose(psb[:, 0:w], xb_t[0:w, :], ident[0:w, 0:w])
                nc.vector.tensor_copy(xbT[:, 128 * t:128 * t + w], psb[:, 0:w])

            def gemm_range(base, w, j):
                """one gate's input GEMM over tokens [base, base+w) + evac"""
                psg = ps_mm.tile([128, 512], F32, tag="psg")
                nc.tensor.matmul(
                    psg[:, 0:w], wx_sb[:, 128 * j:128 * (j + 1)],
                    xaT[:, base:base + w], start=True, stop=False)
                nc.tensor.matmul(
                    psg[:, 0:w], wx_sb[:, 512 + 128 * j:512 + 128 * (j + 1)],
                    xbT[:, base:base + w], start=False, stop=True)
                dst = xgb[:, 4 * base: 4 * base + 32 * (w // 8)] \
                    .rearrange("p (q x) -> p q x", x=32)[:, :, 8 * j:8 * j + 8]
                src = psg[:, 0:w].rearrange("p (q x) -> p q x", x=8)
                nc.vector.tensor_scalar(dst, src, bias_sb[:, j:j + 1], None,
                                        op0=OP.add)

            def gather_wb(t):
                w = _out_w(t)
                nc.gpsimd.indirect_dma_start(
                    out=wgb_all[0:w, WB * t:WB * (t + 1)], out_offset=None,
                    in_=wb[:],
                    in_offset=bass.IndirectOffsetOnAxis(
                        ap=ixm1[0:w, t:t + 1], axis=0))

            def out_tile(t):
                '''prob = sigmoid(h . W_out[idx] + b_out[idx]) * (X != 0)'''
                w = _out_w(t)
                pst = ps_tr.tile([128, 128], F32, tag="psa")
                nc.tensor.transpose(pst[0:w, :], hseq[:, 128 * t:128 * t + w],
                                    ident[:])
                hw_t = wp.tile([128, 128], F32, tag="hw")
                d_t = wp.tile([128, 1], F32, tag="d")
                nc.vector.tensor_tensor(out=hw_t[0:w, :], in0=pst[0:w, :],
                                        in1=wgb_all[0:w, WB * t:WB * t + H],
                                        op=OP.mult)
                nc.vector.tensor_reduce(d_t[0:w, :], hw_t[0:w, :],
                                        axis=mybir.AxisListType.X, op=OP.add)
                p_t = wp.tile([128, 1], F32, tag="p")
                nc.scalar.activation(p_t[0:w, :], d_t[0:w, :], AF.Sigmoid,
                                     bias=wgb_all[0:w, WB * t + H:WB * (t + 1)])
                nc.vector.tensor_tensor(out=prob_sb[0:w, t:t + 1],
                                        in0=p_t[0:w, :],
                                        in1=mnz[0:w, t:t + 1], op=OP.mult)

            # interleave schedule: step index -> list of thunks.
            # chunk c tokens start at step 16*_CHUNKS[c][0]; stay ahead of it.
            side = {}
            tile_steps = {1: (1, 3, 5, 7), 2: (20, 24, 28, 32),
                          3: (52, 56, 60, 64)}
            gemm_steps = {1: (8, 10, 12, 14), 2: (36, 40, 44, 48),
                          3: (68, 72, 76, 80)}
            for j in range(4):                  # second half of tile 0
                side.setdefault(2 + j, []).append(("gemm0b", j))
            for c in (1, 2, 3):
                for k in range(4):
                    side.setdefault(tile_steps[c][k], []).append(
                        ("tile", _CHUNKS[c][0] + k))
                for j in range(4):
                    side.setdefault(gemm_steps[c][j], []).append(("gemm", c, j))
            for t in range(NTILES):             # wgb gathers
                side.setdefault(84 + 4 * t, []).append(("wb", t))
            late_out = []
            for t in range(NTILES):             # output tiles once h is ready
                step = max(16 * t + 17, 140 + 4 * t)
                if step <= S - 1:
                    side.setdefault(step, []).append(("out", t))
                else:
                    late_out.append(t)

            # ---- chunk 0 (first 64 tokens) then the recurrence ----
            process_tile(0)
            for j in range(4):
                gemm_range(0, 64, j)

            for t in range(S):
                psr = ps_rec.tile([128, 32], F32, tag="psr")
                nc.tensor.matmul(psr[:], ident[:], xgb[:, 32 * t:32 * t + 32],
                                 start=True, stop=(t == 0))
                if t > 0:
                    hprev = hseq[:, 8 * (t - 1):8 * t]
                    for j in range(4):
                        nc.tensor.matmul(
                            psr[:, 8 * j:8 * j + 8],
                            whh_sb[:, 128 * j:128 * (j + 1)], hprev,
                            start=False, stop=(j == 3))
                # cols [i|f|o|g]; g was prescaled x2 => tanh(g) = 2*sig-1
                sig = rp.tile([128, 32], F32, tag="sig")
                nc.scalar.activation(sig[:], psr[:], AF.Sigmoid)
                # u = si*(2*sg-1) = 2*w, w = (sg-0.5)*si ; c = f*c + 2w
                w_t = rp.tile([128, 8], F32, tag="w_t")
                if t == 0:
                    nc.vector.scalar_tensor_tensor(
                        out=w_t[:], in0=sig[:, 24:32], scalar=0.5,
                        in1=sig[:, 0:8], op0=OP.subtract, op1=OP.mult)
                    nc.vector.tensor_scalar(c_st[:], w_t[:], 2.0, None,
                                            op0=OP.mult)
                else:
                    cf = rp.tile([128, 8], F32, tag="cf")
                    nc.vector.tensor_tensor(out=cf[:], in0=sig[:, 8:16],
                                            in1=c_st[:], op=OP.mult)
                    nc.vector.scalar_tensor_tensor(
                        out=w_t[:], in0=sig[:, 24:32], scalar=0.5,
                        in1=sig[:, 0:8], op0=OP.subtract, op1=OP.mult)
                    nc.vector.scalar_tensor_tensor(
                        out=c_st[:], in0=w_t[:], scalar=2.0, in1=cf[:],
                        op0=OP.mult, op1=OP.add)
                tch = rp.tile([128, 8], F32, tag="tch")
                nc.scalar.activation(tch[:], c_st[:], AF.Tanh)
                nc.vector.tensor_tensor(out=hseq[:, 8 * t:8 * t + 8],
                                        in0=sig[:, 16:24], in1=tch[:], op=OP.mult)

                for item in side.get(t, ()):
                    if item[0] == "tile":
                        process_tile(item[1])
                    elif item[0] == "gemm0b":
                        gemm_range(64, 64, item[1])
                    elif item[0] == "gemm":
                        t0, ntl = _CHUNKS[item[1]]
                        gemm_range(128 * t0, min(128 * ntl, NT - 128 * t0),
                                   item[2])
                    elif item[0] == "wb":
                        gather_wb(item[1])
                    else:
                        out_tile(item[1])

            for t in late_out:
                out_tile(t)

            nc.sync.dma_start(
                prob[:].rearrange("(t p) -> p t", p=128), prob_sb[:])

    nc.compile()
    return nc


_CACHED = None


def _get_kernel():
    global _CACHED
    if _CACHED is None:
        _CACHED = build_kernel()
    return _CACHED


def _prep_shared(pro_embed, W_ih, W_hh, b_ih, b_hh, W_out, b_out):
    wx_h = np.empty((128, 1024), np.float32)
    whh_h = np.empty((128, 512), np.float32)
    bias_h = np.empty((128, 4), np.float32)
    for j, g in enumerate(_GATE_SRC):
        blk = slice(g * 128, (g + 1) * 128)
        sc = 2.0 if j == 3 else 1.0   # g-gate prescale: tanh(x)=2*sig(2x)-1
        wx_h[:, j * 128:(j + 1) * 128] = sc * W_ih[blk, 0:128].T
        wx_h[:, 512 + j * 128:512 + (j + 1) * 128] = sc * W_ih[blk, 128:256].T
        whh_h[:, j * 128:(j + 1) * 128] = sc * W_hh[blk, :].T
        bias_h[:, j] = sc * (b_ih[blk] + b_hh[blk])
    wb_h = np.empty((PRO_NUM, WB), np.float32)
    wb_h[:, :H] = W_out
    wb_h[:, H] = b_out
    return dict(
        emb=np.ascontiguousarray(pro_embed, np.float32),
        wx=np.ascontiguousarray(wx_h),
        whh=np.ascontiguousarray(whh_h),
        bsum=np.ascontiguousarray(bias_h),
        wb=wb_h,
    )


def kernel(X, y, pro_embed, W_ih, W_hh, b_ih, b_hh, W_out, b_out, _trace=False,
           **_):
    X = np.asarray(X, np.int32)
    y = np.asarray(y, np.int32)
    shared = _prep_shared(np.asarray(pro_embed, np.float32),
                          np.asarray(W_ih, np.float32),
                          np.asarray(W_hh, np.float32),
                          np.asarray(b_ih, np.float32),
                          np.asarray(b_hh, np.float32),
                          np.asarray(W_out, np.float32),
                          np.asarray(b_out, np.float32))
    XT = X.T  # [200, 64]
    YT = y.T
    in_maps = []
    for c in range(N_CORES):
        cols = slice(c * BS, (c + 1) * BS)
        xtp = np.zeros((209, BS), np.int32)
        xtp[:S] = XT[:, cols]
        ytp = np.zeros((208, BS), np.int32)
        ytp[:S] = YT[:, cols]
        in_maps.append(dict(xt=xtp, yt=ytp, **shared))

    nc = _get_kernel()
    res = run_bass_kernel_spmd(nc, in_maps, core_ids=list(range(N_CORES)),
                               trace=_trace)
    out = np.empty((B, S - 1), np.float32)
    for c in range(N_CORES):
        flat = res.results[c]["prob"][:NOUT].reshape(S - 1, BS)
        out[c * BS:(c + 1) * BS, :] = flat.T
    if _trace:
        return out, res
    return out


# revision 16
# speedup vs baseline: 254.9233x; 1.0041x over previous
"""DKT-PEBG kernel for Trainium2 (8 NeuronCores, batch-parallel).

Model: embedding lookup -> masked concat -> LSTM(128) -> per-token output
probability via gathered W_out rows (avoids materializing [B,S,10000]).

Sharding: data-parallel over batch. Core c handles batch rows [8c, 8c+8).
No collectives; host splits inputs / concatenates outputs.

Shapes (hardcoded): B=64, S=200, E=H=128, PRO_NUM=10000.

Recurrence trick: gate order [i,f,o,g] with the g-gate pre-activation
prescaled by 2 on the host, so one Sigmoid over all 4 gates suffices:
tanh(g) = 2*sigmoid(2g) - 1. Input-GEMM chunks and gathers are emitted
interleaved with the first recurrence steps so the scheduler pipelines
them instead of serializing ~45us of startup.
"""

import numpy as np

import concourse.bass as bass
import concourse.bacc as bacc
import concourse.mybir as mybir
import concourse.tile as tile
from concourse.bass_utils import run_bass_kernel_spmd
from concourse.masks import make_identity

B, S = 64, 200
E = 128
H = 128
PRO_NUM = 10000
N_CORES = 8
BS = B // N_CORES              # 8 batch rows per core
NT = BS * S                    # 1600 tokens per core, token n = 8*s + b
NTILES = 13                    # ceil(1600/128); tile 12 has 64 valid tokens
NOUT = BS * (S - 1)            # 1592 output tokens
WB = H + 1                     # gathered W_out row + bias
F32 = mybir.dt.float32
I32 = mybir.dt.int32

_GATE_SRC = (0, 1, 3, 2)       # col blocks [i, f, o, g] <- W_ih row blocks (i,f,g,o)

# input-GEMM chunks in tiles: (first_tile, n_tiles); chunk 0 small so the
# recurrence can start early
_CHUNKS = ((0, 1), (1, 4), (5, 4), (9, 4))


def _tok_w(t):
    return 128 if t < NTILES - 1 else NT - 128 * (NTILES - 1)


def _out_w(t):
    return 128 if t < NTILES - 1 else NOUT - 128 * (NTILES - 1)


def build_kernel():
    nc = bacc.Bacc("TRN2", target_bir_lowering=False, debug=False,
                   num_devices=N_CORES)

    # ---- I/O ----
    xt = nc.dram_tensor("xt", [209, BS], I32, kind="ExternalInput")   # X.T slice, padded
    yt = nc.dram_tensor("yt", [208, BS], I32, kind="ExternalInput")   # y.T slice, padded
    emb = nc.dram_tensor("emb", [PRO_NUM, E], F32, kind="ExternalInput")
    wx = nc.dram_tensor("wx", [128, 1024], F32, kind="ExternalInput")  # W_ih.T blocks [A|B]
    whh = nc.dram_tensor("whh", [128, 512], F32, kind="ExternalInput")  # W_hh.T blocks
    bsum = nc.dram_tensor("bsum", [128, 4], F32, kind="ExternalInput")  # b_ih+b_hh blocks
    wb = nc.dram_tensor("wb", [PRO_NUM, WB], F32, kind="ExternalInput")  # [W_out | b_out]
    prob = nc.dram_tensor("prob", [NTILES * 128], F32, kind="ExternalOutput")

    AF = mybir.ActivationFunctionType
    OP = mybir.AluOpType

    with tile.TileContext(nc) as tc:
        with (
            tc.tile_pool(name="persist", bufs=1) as pp,
            tc.tile_pool(name="work", bufs=3) as wp,
            tc.tile_pool(name="rec", bufs=3) as rp,
            tc.tile_pool(name="ps_tr", bufs=2, space="PSUM") as ps_tr,
            tc.tile_pool(name="ps_mm", bufs=3, space="PSUM") as ps_mm,
            tc.tile_pool(name="ps_rec", bufs=3, space="PSUM") as ps_rec,
        ):
            # ---- persistent SBUF ----
            ident = pp.tile([128, 128], F32, tag="ident")
            wx_sb = pp.tile([128, 1024], F32, tag="wx_sb")
            whh_sb = pp.tile([128, 512], F32, tag="whh_sb")
            bias_sb = pp.tile([128, 4], F32, tag="bias_sb")
            ix_all = pp.tile([128, NTILES], I32, tag="ix_all")
            ixs_all = pp.tile([128, NTILES], I32, tag="ixs_all")
            y_all = pp.tile([128, NTILES], I32, tag="y_all")
            y_f = pp.tile([128, NTILES], F32, tag="y_f")
            m1 = pp.tile([128, NTILES], F32, tag="m1")
            m2 = pp.tile([128, NTILES], F32, tag="m2")
            ixm1 = pp.tile([128, NTILES], I32, tag="ixm1")
            ixs_f = pp.tile([128, NTILES], F32, tag="ixs_f")
            mnz = pp.tile([128, NTILES], F32, tag="mnz")
            xaT = pp.tile([128, NT], F32, tag="xaT")
            xbT = pp.tile([128, NT], F32, tag="xbT")
            xgb = pp.tile([128, 32 * S], F32, tag="xgb")
            hseq = pp.tile([128, NT], F32, tag="hseq")
            wgb_all = pp.tile([128, NTILES * WB], F32, tag="wgb_all")
            c_st = pp.tile([128, BS], F32, tag="c_st")
            prob_sb = pp.tile([128, NTILES], F32, tag="prob_sb")

            make_identity(nc, ident[:])
            nc.gpsimd.memset(prob_sb[:], 0.0)

            # ---- loads ----
            xt_flat = xt[:].rearrange("s b -> (s b)")
            yt_flat = yt[:].rearrange("s b -> (s b)")
            nc.sync.dma_start(
                ix_all[:], xt_flat[0:1664].rearrange("(t p) -> p t", p=128))
            nc.sync.dma_start(
                y_all[:], yt_flat[0:1664].rearrange("(t p) -> p t", p=128))
            nc.sync.dma_start(
                ixs_all[:], xt_flat[8:1672].rearrange("(t p) -> p t", p=128))
            nc.sync.dma_start(wx_sb[:], wx[:])
            nc.sync.dma_start(whh_sb[:], whh[:])
            nc.sync.dma_start(bias_sb[:], bsum[:])

            # warm the ACT sigmoid/tanh table set off the critical path
            warm = wp.tile([1, 1], F32, tag="warm")
            nc.scalar.activation(warm[:], ident[0:1, 0:1], AF.Sigmoid)

            # masks: m1 = (y==0), m2 = (y==1); padding y==-1 -> 0,0
            nc.vector.tensor_copy(y_f[:], y_all[:])
            nc.vector.tensor_scalar(m1[:], y_f[:], 0.0, None, op0=OP.is_equal)
            nc.vector.tensor_scalar(m2[:], y_f[:], 1.0, None, op0=OP.is_equal)
            # ixm1 = max(X[s+1]-1, 0), mnz = (X[s+1] != 0)
            nc.vector.tensor_scalar(ixm1[:], ixs_all[:], 1, 0,
                                    op0=OP.subtract, op1=OP.max)
            nc.vector.tensor_copy(ixs_f[:], ixs_all[:])
            nc.vector.tensor_scalar(mnz[:], ixs_f[:], 0.0, None,
                                    op0=OP.not_equal)

            def process_tile(t):
                """gather embeddings for tile t, mask, transpose into xaT/xbT"""
                w = _tok_w(t)
                ex_t = wp.tile([128, E], F32, tag="ex")
                nc.gpsimd.indirect_dma_start(
                    out=ex_t[0:w, :], out_offset=None, in_=emb[:],
                    in_offset=bass.IndirectOffsetOnAxis(
                        ap=ix_all[0:w, t:t + 1], axis=0))
                xa_t = wp.tile([128, E], F32, tag="xa")
                xb_t = wp.tile([128, E], F32, tag="xb")
                nc.vector.tensor_scalar(xa_t[0:w, :], ex_t[0:w, :],
                                        m1[0:w, t:t + 1], None, op0=OP.mult)
                nc.vector.tensor_scalar(xb_t[0:w, :], ex_t[0:w, :],
                                        m2[0:w, t:t + 1], None, op0=OP.mult)
                psa = ps_tr.tile([128, 128], F32, tag="psa")
                nc.tensor.transpose(psa[:, 0:w], xa_t[0:w, :], ident[0:w, 0:w])
                nc.vector.tensor_copy(xaT[:, 128 * t:128 * t + w], psa[:, 0:w])
                psb = ps_tr.tile([128, 128], F32, tag="psa")
                nc.tensor.transpose(psb[:, 0:w], xb_t[0:w, :], ident[0:w, 0:w])
                nc.vector.tensor_copy(xbT[:, 128 * t:128 * t + w], psb[:, 0:w])

            pending_psg = {}

            def gemm_a(base, w, j):
                psg = ps_mm.tile([128, 512], F32, tag="psg")
                pending_psg[(base, j)] = psg
                nc.tensor.matmul(
                    psg[:, 0:w], wx_sb[:, 128 * j:128 * (j + 1)],
                    xaT[:, base:base + w], start=True, stop=False)

            def gemm_b(base, w, j):
                psg = pending_psg.pop((base, j))
                nc.tensor.matmul(
                    psg[:, 0:w], wx_sb[:, 512 + 128 * j:512 + 128 * (j + 1)],
                    xbT[:, base:base + w], start=False, stop=True)
                dst = xgb[:, 4 * base: 4 * base + 32 * (w // 8)] \
                    .rearrange("p (q x) -> p q x", x=32)[:, :, 8 * j:8 * j + 8]
                src = psg[:, 0:w].rearrange("p (q x) -> p q x", x=8)
                nc.vector.tensor_scalar(dst, src, bias_sb[:, j:j + 1], None,
                                        op0=OP.add)

            def gemm_range(base, w, j):
                gemm_a(base, w, j)
                gemm_b(base, w, j)

            def gather_wb(t):
                w = _out_w(t)
                nc.gpsimd.indirect_dma_start(
                    out=wgb_all[0:w, WB * t:WB * (t + 1)], out_offset=None,
                    in_=wb[:],
                    in_offset=bass.IndirectOffsetOnAxis(
                        ap=ixm1[0:w, t:t + 1], axis=0))

            def out_tile(t):
                '''prob = sigmoid(h . W_out[idx] + b_out[idx]) * (X != 0)'''
                w = _out_w(t)
                pst = ps_tr.tile([128, 128], F32, tag="psa")
                nc.tensor.transpose(pst[0:w, :], hseq[:, 128 * t:128 * t + w],
                                    ident[:])
                hw_t = wp.tile([128, 128], F32, tag="hw")
                d_t = wp.tile([128, 1], F32, tag="d")
                nc.vector.tensor_tensor(out=hw_t[0:w, :], in0=pst[0:w, :],
                                        in1=wgb_all[0:w, WB * t:WB * t + H],
                                        op=OP.mult)
                nc.vector.tensor_reduce(d_t[0:w, :], hw_t[0:w, :],
                                        axis=mybir.AxisListType.X, op=OP.add)
                p_t = wp.tile([128, 1], F32, tag="p")
                nc.scalar.activation(p_t[0:w, :], d_t[0:w, :], AF.Sigmoid,
                                     bias=wgb_all[0:w, WB * t + H:WB * (t + 1)])
                nc.vector.tensor_tensor(out=prob_sb[0:w, t:t + 1],
                                        in0=p_t[0:w, :],
                                        in1=mnz[0:w, t:t + 1], op=OP.mult)

            # interleave schedule: step index -> list of thunks.
            # chunk c tokens start at step 16*_CHUNKS[c][0]; stay ahead of it.
            side = {}
            tile_steps = {1: (1, 2, 3, 4), 2: (20, 24, 28, 32),
                          3: (52, 56, 60, 64)}
            gemm_steps = {1: 5, 2: 36, 3: 68}   # first of 8 alternating slots
            for j in range(4):                  # second half of tile 0
                side.setdefault(2 + j, []).append(("gemm0b", j))
            for c in (1, 2, 3):
                for k in range(4):
                    side.setdefault(tile_steps[c][k], []).append(
                        ("tile", _CHUNKS[c][0] + k))
                for j in range(4):
                    s0 = gemm_steps[c] + 2 * j
                    side.setdefault(s0, []).append(("gemm_a", c, j))
                    side.setdefault(s0 + 1, []).append(("gemm_b", c, j))
            for t in range(NTILES):             # wgb gathers
                side.setdefault(84 + 4 * t, []).append(("wb", t))
            late_out = []
            for t in range(NTILES):             # output tiles once h is ready
                step = max(16 * t + 17, 140 + 4 * t)
                if step <= S - 1:
                    side.setdefault(step, []).append(("out", t))
                else:
                    late_out.append(t)
            side.setdefault(196, []).append(("probdma",))

            # ---- chunk 0 (first 64 tokens) then the recurrence ----
            process_tile(0)
            for j in range(4):
                gemm_range(0, 64, j)

            for t in range(S):
                psr = ps_rec.tile([128, 32], F32, tag="psr")
                nc.tensor.matmul(psr[:], ident[:], xgb[:, 32 * t:32 * t + 32],
                                 start=True, stop=(t == 0))
                if t > 0:
                    hprev = hseq[:, 8 * (t - 1):8 * t]
                    for j in range(4):
                        nc.tensor.matmul(
                            psr[:, 8 * j:8 * j + 8],
                            whh_sb[:, 128 * j:128 * (j + 1)], hprev,
                            start=False, stop=(j == 3))
                # cols [i|f|o|g]; g was prescaled x2 => tanh(g) = 2*sig-1
                sig = rp.tile([128, 32], F32, tag="sig")
                nc.scalar.activation(sig[:], psr[:], AF.Sigmoid)
                # u = si*(2*sg-1) = 2*w, w = (sg-0.5)*si ; c = f*c + 2w
                w_t = rp.tile([128, 8], F32, tag="w_t")
                if t == 0:
                    nc.vector.scalar_tensor_tensor(
                        out=w_t[:], in0=sig[:, 24:32], scalar=0.5,
                        in1=sig[:, 0:8], op0=OP.subtract, op1=OP.mult)
                    nc.vector.tensor_scalar(c_st[:], w_t[:], 2.0, None,
                                            op0=OP.mult)
                else:
                    cf = rp.tile([128, 8], F32, tag="cf")
                    nc.vector.tensor_tensor(out=cf[:], in0=sig[:, 8:16],
                                            in1=c_st[:], op=OP.mult)
                    nc.vector.scalar_tensor_tensor(
                        out=w_t[:], in0=sig[:, 24:32], scalar=0.5,
                        in1=sig[:, 0:8], op0=OP.subtract, op1=OP.mult)
                    nc.vector.scalar_tensor_tensor(
                        out=c_st[:], in0=w_t[:], scalar=2.0, in1=cf[:],
                        op0=OP.mult, op1=OP.add)
                tch = rp.tile([128, 8], F32, tag="tch")
                nc.scalar.activation(tch[:], c_st[:], AF.Tanh)
                nc.vector.tensor_tensor(out=hseq[:, 8 * t:8 * t + 8],
                                        in0=sig[:, 16:24], in1=tch[:], op=OP.mult)

                for item in side.get(t, ()):
                    if item[0] == "tile":
                        process_tile(item[1])
                    elif item[0] == "gemm0b":
                        gemm_range(64, 64, item[1])
                    elif item[0] in ("gemm_a", "gemm_b"):
                        t0, ntl = _CHUNKS[item[1]]
                        fn = gemm_a if item[0] == "gemm_a" else gemm_b
                        fn(128 * t0, min(128 * ntl, NT - 128 * t0), item[2])
                    elif item[0] == "probdma":
                        nc.sync.dma_start(
                            prob[:].rearrange("(t p) -> p t", p=128)[:, 0:12],
                            prob_sb[:, 0:12])
                    elif item[0] == "wb":
                        gather_wb(item[1])
                    else:
                        out_tile(item[1])

            for t in late_out:
                out_tile(t)

            nc.sync.dma_start(
                prob[:].rearrange("(t p) -> p t", p=128)[:, 12:13],
                prob_sb[:, 12:13])

    nc.compile()
    return nc


_CACHED = None


def _get_kernel():
    global _CACHED
    if _CACHED is None:
        _CACHED = build_kernel()
    return _CACHED


def _prep_shared(pro_embed, W_ih, W_hh, b_ih, b_hh, W_out, b_out):
    wx_h = np.empty((128, 1024), np.float32)
    whh_h = np.empty((128, 512), np.float32)
    bias_h = np.empty((128, 4), np.float32)
    for j, g in enumerate(_GATE_SRC):
        blk = slice(g * 128, (g + 1) * 128)
        sc = 2.0 if j == 3 else 1.0   # g-gate prescale: tanh(x)=2*sig(2x)-1
        wx_h[:, j * 128:(j + 1) * 128] = sc * W_ih[blk, 0:128].T
        wx_h[:, 512 + j * 128:512 + (j + 1) * 128] = sc * W_ih[blk, 128:256].T
        whh_h[:, j * 128:(j + 1) * 128] = sc * W_hh[blk, :].T
        bias_h[:, j] = sc * (b_ih[blk] + b_hh[blk])
    wb_h = np.empty((PRO_NUM, WB), np.float32)
    wb_h[:, :H] = W_out
    wb_h[:, H] = b_out
    return dict(
        emb=np.ascontiguousarray(pro_embed, np.float32),
        wx=np.ascontiguousarray(wx_h),
        whh=np.ascontiguousarray(whh_h),
        bsum=np.ascontiguousarray(bias_h),
        wb=wb_h,
    )


def kernel(X, y, pro_embed, W_ih, W_hh, b_ih, b_hh, W_out, b_out, _trace=False,
           **_):
    X = np.asarray(X, np.int32)
    y = np.asarray(y, np.int32)
    shared = _prep_shared(np.asarray(pro_embed, np.float32),
                          np.asarray(W_ih, np.float32),
                          np.asarray(W_hh, np.float32),
                          np.asarray(b_ih, np.float32),
                          np.asarray(b_hh, np.float32),
                          np.asarray(W_out, np.float32),
                          np.asarray(b_out, np.float32))
    XT = X.T  # [200, 64]
    YT = y.T
    in_maps = []
    for c in range(N_CORES):
        cols = slice(c * BS, (c + 1) * BS)
        xtp = np.zeros((209, BS), np.int32)
        xtp[:S] = XT[:, cols]
        ytp = np.zeros((208, BS), np.int32)
        ytp[:S] = YT[:, cols]
        in_maps.append(dict(xt=xtp, yt=ytp, **shared))

    nc = _get_kernel()
    res = run_bass_kernel_spmd(nc, in_maps, core_ids=list(range(N_CORES)),
                               trace=_trace)
    out = np.empty((B, S - 1), np.float32)
    for c in range(N_CORES):
        flat = res.results[c]["prob"][:NOUT].reshape(S - 1, BS)
        out[c * BS:(c + 1) * BS, :] = flat.T
    if _trace:
        return out, res
    return out


# revision 19
# speedup vs baseline: 255.1737x; 1.0010x over previous
"""DKT-PEBG kernel for Trainium2 (8 NeuronCores, batch-parallel).

Model: embedding lookup -> masked concat -> LSTM(128) -> per-token output
probability via gathered W_out rows (avoids materializing [B,S,10000]).

Sharding: data-parallel over batch. Core c handles batch rows [8c, 8c+8).
No collectives; host splits inputs / concatenates outputs.

Shapes (hardcoded): B=64, S=200, E=H=128, PRO_NUM=10000.

Recurrence trick: gate order [i,f,o,g] with the g-gate pre-activation
prescaled by 2 on the host, so one Sigmoid over all 4 gates suffices:
tanh(g) = 2*sigmoid(2g) - 1. Input-GEMM chunks and gathers are emitted
interleaved with the first recurrence steps so the scheduler pipelines
them instead of serializing ~45us of startup.
"""

import numpy as np

import concourse.bass as bass
import concourse.bacc as bacc
import concourse.mybir as mybir
import concourse.tile as tile
from concourse.bass_utils import run_bass_kernel_spmd
from concourse.masks import make_identity

B, S = 64, 200
E = 128
H = 128
PRO_NUM = 10000
N_CORES = 8
BS = B // N_CORES              # 8 batch rows per core
NT = BS * S                    # 1600 tokens per core, token n = 8*s + b
NTILES = 13                    # ceil(1600/128); tile 12 has 64 valid tokens
NOUT = BS * (S - 1)            # 1592 output tokens
WB = H + 1                     # gathered W_out row + bias
F32 = mybir.dt.float32
I32 = mybir.dt.int32

_GATE_SRC = (0, 1, 3, 2)       # col blocks [i, f, o, g] <- W_ih row blocks (i,f,g,o)

# input-GEMM chunks in tiles: (first_tile, n_tiles); chunk 0 small so the
# recurrence can start early
_CHUNKS = ((0, 1), (1, 4), (5, 4), (9, 4))


def _tok_w(t):
    return 128 if t < NTILES - 1 else NT - 128 * (NTILES - 1)


def _out_w(t):
    return 128 if t < NTILES - 1 else NOUT - 128 * (NTILES - 1)


def build_kernel():
    nc = bacc.Bacc("TRN2", target_bir_lowering=False, debug=False,
                   num_devices=N_CORES)

    # ---- I/O ----
    xt = nc.dram_tensor("xt", [209, BS], I32, kind="ExternalInput")   # X.T slice, padded
    yt = nc.dram_tensor("yt", [208, BS], I32, kind="ExternalInput")   # y.T slice, padded
    emb = nc.dram_tensor("emb", [PRO_NUM, E], F32, kind="ExternalInput")
    wx = nc.dram_tensor("wx", [128, 1024], F32, kind="ExternalInput")  # W_ih.T blocks [A|B]
    whh = nc.dram_tensor("whh", [128, 512], F32, kind="ExternalInput")  # W_hh.T blocks
    bsum = nc.dram_tensor("bsum", [128, 4], F32, kind="ExternalInput")  # b_ih+b_hh blocks
    wb = nc.dram_tensor("wb", [PRO_NUM, WB], F32, kind="ExternalInput")  # [W_out | b_out]
    prob = nc.dram_tensor("prob", [NTILES * 128], F32, kind="ExternalOutput")

    AF = mybir.ActivationFunctionType
    OP = mybir.AluOpType

    with tile.TileContext(nc) as tc:
        with (
            tc.tile_pool(name="persist", bufs=1) as pp,
            tc.tile_pool(name="work", bufs=4) as wp,
            tc.tile_pool(name="rec", bufs=4) as rp,
            tc.tile_pool(name="ps_tr", bufs=2, space="PSUM") as ps_tr,
            tc.tile_pool(name="ps_mm", bufs=3, space="PSUM") as ps_mm,
            tc.tile_pool(name="ps_rec", bufs=3, space="PSUM") as ps_rec,
        ):
            # ---- persistent SBUF ----
            ident = pp.tile([128, 128], F32, tag="ident")
            wx_sb = pp.tile([128, 1024], F32, tag="wx_sb")
            whh_sb = pp.tile([128, 512], F32, tag="whh_sb")
            bias_sb = pp.tile([128, 4], F32, tag="bias_sb")
            ix_all = pp.tile([128, NTILES], I32, tag="ix_all")
            ixs_all = pp.tile([128, NTILES], I32, tag="ixs_all")
            y_all = pp.tile([128, NTILES], I32, tag="y_all")
            y_f = pp.tile([128, NTILES], F32, tag="y_f")
            m1 = pp.tile([128, NTILES], F32, tag="m1")
            m2 = pp.tile([128, NTILES], F32, tag="m2")
            ixm1 = pp.tile([128, NTILES], I32, tag="ixm1")
            ixs_f = pp.tile([128, NTILES], F32, tag="ixs_f")
            mnz = pp.tile([128, NTILES], F32, tag="mnz")
            xaT = pp.tile([128, NT], F32, tag="xaT")
            xbT = pp.tile([128, NT], F32, tag="xbT")
            xgb = pp.tile([128, 32 * S], F32, tag="xgb")
            hseq = pp.tile([128, NT], F32, tag="hseq")
            wgb_all = pp.tile([128, NTILES * WB], F32, tag="wgb_all")
            c_st = pp.tile([128, BS], F32, tag="c_st")
            prob_sb = pp.tile([128, NTILES], F32, tag="prob_sb")

            make_identity(nc, ident[:])
            nc.gpsimd.memset(prob_sb[:], 0.0)

            # ---- loads ----
            xt_flat = xt[:].rearrange("s b -> (s b)")
            yt_flat = yt[:].rearrange("s b -> (s b)")
            nc.sync.dma_start(
                ix_all[:], xt_flat[0:1664].rearrange("(t p) -> p t", p=128))
            nc.sync.dma_start(
                y_all[:], yt_flat[0:1664].rearrange("(t p) -> p t", p=128))
            nc.sync.dma_start(
                ixs_all[:], xt_flat[8:1672].rearrange("(t p) -> p t", p=128))
            nc.sync.dma_start(wx_sb[:], wx[:])
            nc.sync.dma_start(whh_sb[:], whh[:])
            nc.sync.dma_start(bias_sb[:], bsum[:])

            # warm the ACT sigmoid/tanh table set off the critical path
            warm = wp.tile([1, 1], F32, tag="warm")
            nc.scalar.activation(warm[:], ident[0:1, 0:1], AF.Sigmoid)

            # masks: m1 = (y==0), m2 = (y==1); padding y==-1 -> 0,0
            nc.vector.tensor_copy(y_f[:], y_all[:])
            nc.vector.tensor_scalar(m1[:], y_f[:], 0.0, None, op0=OP.is_equal)
            nc.vector.tensor_scalar(m2[:], y_f[:], 1.0, None, op0=OP.is_equal)
            # ixm1 = max(X[s+1]-1, 0), mnz = (X[s+1] != 0)
            nc.vector.tensor_scalar(ixm1[:], ixs_all[:], 1, 0,
                                    op0=OP.subtract, op1=OP.max)
            nc.vector.tensor_copy(ixs_f[:], ixs_all[:])
            nc.vector.tensor_scalar(mnz[:], ixs_f[:], 0.0, None,
                                    op0=OP.not_equal)

            def process_tile(t):
                """gather embeddings for tile t, mask, transpose into xaT/xbT"""
                w = _tok_w(t)
                ex_t = wp.tile([128, E], F32, tag="ex")
                nc.gpsimd.indirect_dma_start(
                    out=ex_t[0:w, :], out_offset=None, in_=emb[:],
                    in_offset=bass.IndirectOffsetOnAxis(
                        ap=ix_all[0:w, t:t + 1], axis=0))
                xa_t = wp.tile([128, E], F32, tag="xa")
                xb_t = wp.tile([128, E], F32, tag="xb")
                nc.vector.tensor_scalar(xa_t[0:w, :], ex_t[0:w, :],
                                        m1[0:w, t:t + 1], None, op0=OP.mult)
                nc.vector.tensor_scalar(xb_t[0:w, :], ex_t[0:w, :],
                                        m2[0:w, t:t + 1], None, op0=OP.mult)
                psa = ps_tr.tile([128, 128], F32, tag="psa")
                nc.tensor.transpose(psa[:, 0:w], xa_t[0:w, :], ident[0:w, 0:w])
                nc.vector.tensor_copy(xaT[:, 128 * t:128 * t + w], psa[:, 0:w])
                psb = ps_tr.tile([128, 128], F32, tag="psa")
                nc.tensor.transpose(psb[:, 0:w], xb_t[0:w, :], ident[0:w, 0:w])
                nc.vector.tensor_copy(xbT[:, 128 * t:128 * t + w], psb[:, 0:w])

            pending_psg = {}

            def gemm_a(base, w, j):
                psg = ps_mm.tile([128, 512], F32, tag="psg")
                pending_psg[(base, j)] = psg
                nc.tensor.matmul(
                    psg[:, 0:w], wx_sb[:, 128 * j:128 * (j + 1)],
                    xaT[:, base:base + w], start=True, stop=False)

            def gemm_b(base, w, j):
                psg = pending_psg.pop((base, j))
                nc.tensor.matmul(
                    psg[:, 0:w], wx_sb[:, 512 + 128 * j:512 + 128 * (j + 1)],
                    xbT[:, base:base + w], start=False, stop=True)
                dst = xgb[:, 4 * base: 4 * base + 32 * (w // 8)] \
                    .rearrange("p (q x) -> p q x", x=32)[:, :, 8 * j:8 * j + 8]
                src = psg[:, 0:w].rearrange("p (q x) -> p q x", x=8)
                nc.vector.tensor_scalar(dst, src, bias_sb[:, j:j + 1], None,
                                        op0=OP.add)

            def gemm_range(base, w, j):
                gemm_a(base, w, j)
                gemm_b(base, w, j)

            def gather_wb(t):
                w = _out_w(t)
                nc.gpsimd.indirect_dma_start(
                    out=wgb_all[0:w, WB * t:WB * (t + 1)], out_offset=None,
                    in_=wb[:],
                    in_offset=bass.IndirectOffsetOnAxis(
                        ap=ixm1[0:w, t:t + 1], axis=0))

            def out_tile(t):
                '''prob = sigmoid(h . W_out[idx] + b_out[idx]) * (X != 0)'''
                w = _out_w(t)
                pst = ps_tr.tile([128, 128], F32, tag="psa")
                nc.tensor.transpose(pst[0:w, :], hseq[:, 128 * t:128 * t + w],
                                    ident[:])
                hw_t = wp.tile([128, 128], F32, tag="hw")
                d_t = wp.tile([128, 1], F32, tag="d")
                nc.vector.tensor_tensor(out=hw_t[0:w, :], in0=pst[0:w, :],
                                        in1=wgb_all[0:w, WB * t:WB * t + H],
                                        op=OP.mult)
                nc.vector.tensor_reduce(d_t[0:w, :], hw_t[0:w, :],
                                        axis=mybir.AxisListType.X, op=OP.add)
                p_t = wp.tile([128, 1], F32, tag="p")
                nc.scalar.activation(p_t[0:w, :], d_t[0:w, :], AF.Sigmoid,
                                     bias=wgb_all[0:w, WB * t + H:WB * (t + 1)])
                nc.vector.tensor_tensor(out=prob_sb[0:w, t:t + 1],
                                        in0=p_t[0:w, :],
                                        in1=mnz[0:w, t:t + 1], op=OP.mult)

            # interleave schedule: step index -> list of thunks.
            # chunk c tokens start at step 16*_CHUNKS[c][0]; stay ahead of it.
            side = {}
            tile_steps = {1: (1, 2, 3, 4), 2: (20, 24, 28, 32),
                          3: (52, 56, 60, 64)}
            gemm_steps = {1: 5, 2: 36, 3: 68}   # first of 8 alternating slots
            for j in range(4):                  # second half of tile 0
                side.setdefault(2 + j, []).append(("gemm0b", j))
            for c in (1, 2, 3):
                for k in range(4):
                    side.setdefault(tile_steps[c][k], []).append(
                        ("tile", _CHUNKS[c][0] + k))
                for j in range(4):
                    s0 = gemm_steps[c] + 2 * j
                    side.setdefault(s0, []).append(("gemm_a", c, j))
                    side.setdefault(s0 + 1, []).append(("gemm_b", c, j))
            for t in range(NTILES):             # wgb gathers
                side.setdefault(84 + 4 * t, []).append(("wb", t))
            late_out = []
            for t in range(NTILES):             # output tiles once h is ready
                step = max(16 * t + 17, 140 + 4 * t)
                if step <= S - 1:
                    side.setdefault(step, []).append(("out", t))
                else:
                    late_out.append(t)
            side.setdefault(196, []).append(("probdma",))

            # ---- chunk 0 (first 64 tokens) then the recurrence ----
            process_tile(0)
            for j in range(4):
                gemm_range(0, 64, j)

            for t in range(S):
                psr = ps_rec.tile([128, 32], F32, tag="psr")
                nc.tensor.matmul(psr[:], ident[:], xgb[:, 32 * t:32 * t + 32],
                                 start=True, stop=(t == 0))
                if t > 0:
                    hprev = hseq[:, 8 * (t - 1):8 * t]
                    for j in range(4):
                        nc.tensor.matmul(
                            psr[:, 8 * j:8 * j + 8],
                            whh_sb[:, 128 * j:128 * (j + 1)], hprev,
                            start=False, stop=(j == 3))
                # cols [i|f|o|g]; g was prescaled x2 => tanh(g) = 2*sig-1
                sig = rp.tile([128, 32], F32, tag="sig")
                nc.scalar.activation(sig[:], psr[:], AF.Sigmoid)
                # u = si*(2*sg-1) = 2*w, w = (sg-0.5)*si ; c = f*c + 2w
                w_t = rp.tile([128, 8], F32, tag="w_t")
                if t == 0:
                    nc.vector.scalar_tensor_tensor(
                        out=w_t[:], in0=sig[:, 24:32], scalar=0.5,
                        in1=sig[:, 0:8], op0=OP.subtract, op1=OP.mult)
                    nc.vector.tensor_scalar(c_st[:], w_t[:], 2.0, None,
                                            op0=OP.mult)
                else:
                    cf = rp.tile([128, 8], F32, tag="cf")
                    nc.vector.tensor_tensor(out=cf[:], in0=sig[:, 8:16],
                                            in1=c_st[:], op=OP.mult)
                    nc.vector.scalar_tensor_tensor(
                        out=w_t[:], in0=sig[:, 24:32], scalar=0.5,
                        in1=sig[:, 0:8], op0=OP.subtract, op1=OP.mult)
                    nc.vector.scalar_tensor_tensor(
                        out=c_st[:], in0=w_t[:], scalar=2.0, in1=cf[:],
                        op0=OP.mult, op1=OP.add)
                tch = rp.tile([128, 8], F32, tag="tch")
                nc.scalar.activation(tch[:], c_st[:], AF.Tanh)
                nc.vector.tensor_tensor(out=hseq[:, 8 * t:8 * t + 8],
                                        in0=sig[:, 16:24], in1=tch[:], op=OP.mult)

                for item in side.get(t, ()):
                    if item[0] == "tile":
                        process_tile(item[1])
                    elif item[0] == "gemm0b":
                        gemm_range(64, 64, item[1])
                    elif item[0] in ("gemm_a", "gemm_b"):
                        t0, ntl = _CHUNKS[item[1]]
                        fn = gemm_a if item[0] == "gemm_a" else gemm_b
                        fn(128 * t0, min(128 * ntl, NT - 128 * t0), item[2])
                    elif item[0] == "probdma":
                        nc.sync.dma_start(
                            prob[:].rearrange("(t p) -> p t", p=128)[:, 0:12],
                            prob_sb[:, 0:12])
                    elif item[0] == "wb":
                        gather_wb(item[1])
                    else:
                        out_tile(item[1])

            for t in late_out:
                out_tile(t)

            nc.sync.dma_start(
                prob[:].rearrange("(t p) -> p t", p=128)[:, 12:13],
                prob_sb[:, 12:13])

    nc.compile()
    return nc


_CACHED = None


def _get_kernel():
    global _CACHED
    if _CACHED is None:
        _CACHED = build_kernel()
    return _CACHED


def _prep_shared(pro_embed, W_ih, W_hh, b_ih, b_hh, W_out, b_out):
    wx_h = np.empty((128, 1024), np.float32)
    whh_h = np.empty((128, 512), np.float32)
    bias_h = np.empty((128, 4), np.float32)
    for j, g in enumerate(_GATE_SRC):
        blk = slice(g * 128, (g + 1) * 128)
        sc = 2.0 if j == 3 else 1.0   # g-gate prescale: tanh(x)=2*sig(2x)-1
        wx_h[:, j * 128:(j + 1) * 128] = sc * W_ih[blk, 0:128].T
        wx_h[:, 512 + j * 128:512 + (j + 1) * 128] = sc * W_ih[blk, 128:256].T
        whh_h[:, j * 128:(j + 1) * 128] = sc * W_hh[blk, :].T
        bias_h[:, j] = sc * (b_ih[blk] + b_hh[blk])
    wb_h = np.empty((PRO_NUM, WB), np.float32)
    wb_h[:, :H] = W_out
    wb_h[:, H] = b_out
    return dict(
        emb=np.ascontiguousarray(pro_embed, np.float32),
        wx=np.ascontiguousarray(wx_h),
        whh=np.ascontiguousarray(whh_h),
        bsum=np.ascontiguousarray(bias_h),
        wb=wb_h,
    )


def kernel(X, y, pro_embed, W_ih, W_hh, b_ih, b_hh, W_out, b_out, _trace=False,
           **_):
    X = np.asarray(X, np.int32)
    y = np.asarray(y, np.int32)
    shared = _prep_shared(np.asarray(pro_embed, np.float32),
                          np.asarray(W_ih, np.float32),
                          np.asarray(W_hh, np.float32),
                          np.asarray(b_ih, np.float32),
                          np.asarray(b_hh, np.float32),
                          np.asarray(W_out, np.float32),
                          np.asarray(b_out, np.float32))
    XT = X.T  # [200, 64]
    YT = y.T
    in_maps = []
    for c in range(N_CORES):
        cols = slice(c * BS, (c + 1) * BS)
        xtp = np.zeros((209, BS), np.int32)
        xtp[:S] = XT[:, cols]
        ytp = np.zeros((208, BS), np.int32)
        ytp[:S] = YT[:, cols]
        in_maps.append(dict(xt=xtp, yt=ytp, **shared))

    nc = _get_kernel()
    res = run_bass_kernel_spmd(nc, in_maps, core_ids=list(range(N_CORES)),
                               trace=_trace)
    out = np.empty((B, S - 1), np.float32)
    for c in range(N_CORES):
        flat = res.results[c]["prob"][:NOUT].reshape(S - 1, BS)
        out[c * BS:(c + 1) * BS, :] = flat.T
    if _trace:
        return out, res
    return out


# revision 23
# speedup vs baseline: 261.5305x; 1.0249x over previous
"""DKT-PEBG kernel for Trainium2 (8 NeuronCores, batch-parallel).

Model: embedding lookup -> masked concat -> LSTM(128) -> per-token output
probability via gathered W_out rows (avoids materializing [B,S,10000]).

Sharding: data-parallel over batch. Core c handles batch rows [8c, 8c+8).
No collectives; host splits inputs / concatenates outputs.

Shapes (hardcoded): B=64, S=200, E=H=128, PRO_NUM=10000.

Recurrence trick: gate order [i,f,o,g] with the g-gate pre-activation
prescaled by 2 on the host, so one Sigmoid over all 4 gates suffices:
tanh(g) = 2*sigmoid(2g) - 1. Input-GEMM chunks and gathers are emitted
interleaved with the first recurrence steps so the scheduler pipelines
them instead of serializing ~45us of startup.
"""

import numpy as np

import concourse.bass as bass
import concourse.bacc as bacc
import concourse.mybir as mybir
import concourse.tile as tile
from concourse.bass_utils import run_bass_kernel_spmd
from concourse.masks import make_identity

B, S = 64, 200
E = 128
H = 128
PRO_NUM = 10000
N_CORES = 8
BS = B // N_CORES              # 8 batch rows per core
NT = BS * S                    # 1600 tokens per core, token n = 8*s + b
NTILES = 13                    # ceil(1600/128); tile 12 has 64 valid tokens
NOUT = BS * (S - 1)            # 1592 output tokens
WB = H + 1                     # gathered W_out row + bias
F32 = mybir.dt.float32
I32 = mybir.dt.int32

_GATE_SRC = (0, 1, 3, 2)       # col blocks [i, f, o, g] <- W_ih row blocks (i,f,g,o)

# input-GEMM chunks in tiles: (first_tile, n_tiles); chunk 0 small so the
# recurrence can start early
_CHUNKS = ((0, 1), (1, 2), (3, 2), (5, 4), (9, 4))


def _tok_w(t):
    return 128 if t < NTILES - 1 else NT - 128 * (NTILES - 1)


def _out_w(t):
    return 128 if t < NTILES - 1 else NOUT - 128 * (NTILES - 1)


def build_kernel():
    nc = bacc.Bacc("TRN2", target_bir_lowering=False, debug=False,
                   num_devices=N_CORES)

    # ---- I/O ----
    xt = nc.dram_tensor("xt", [209, BS], I32, kind="ExternalInput")   # X.T slice, padded
    yt = nc.dram_tensor("yt", [208, BS], I32, kind="ExternalInput")   # y.T slice, padded
    emb = nc.dram_tensor("emb", [PRO_NUM, E], F32, kind="ExternalInput")
    wx = nc.dram_tensor("wx", [128, 1024], F32, kind="ExternalInput")  # W_ih.T blocks [A|B]
    whh = nc.dram_tensor("whh", [128, 512], F32, kind="ExternalInput")  # W_hh.T blocks
    bsum = nc.dram_tensor("bsum", [128, 4], F32, kind="ExternalInput")  # b_ih+b_hh blocks
    wb = nc.dram_tensor("wb", [PRO_NUM, WB], F32, kind="ExternalInput")  # [W_out | b_out]
    prob = nc.dram_tensor("prob", [NTILES * 128], F32, kind="ExternalOutput")

    AF = mybir.ActivationFunctionType
    OP = mybir.AluOpType

    with tile.TileContext(nc) as tc:
        with (
            tc.tile_pool(name="persist", bufs=1) as pp,
            tc.tile_pool(name="work", bufs=4) as wp,
            tc.tile_pool(name="exp", bufs=13) as expool,
            tc.tile_pool(name="rec", bufs=4) as rp,
            tc.tile_pool(name="ps_tr", bufs=2, space="PSUM") as ps_tr,
            tc.tile_pool(name="ps_mm", bufs=3, space="PSUM") as ps_mm,
            tc.tile_pool(name="ps_rec", bufs=3, space="PSUM") as ps_rec,
        ):
            # ---- persistent SBUF ----
            ident = pp.tile([128, 128], F32, tag="ident")
            wx_sb = pp.tile([128, 1024], F32, tag="wx_sb")
            whh_sb = pp.tile([128, 512], F32, tag="whh_sb")
            bias_sb = pp.tile([128, 4], F32, tag="bias_sb")
            ix_all = pp.tile([128, NTILES], I32, tag="ix_all")
            ixs_all = pp.tile([128, NTILES], I32, tag="ixs_all")
            y_all = pp.tile([128, NTILES], I32, tag="y_all")
            y_f = pp.tile([128, NTILES], F32, tag="y_f")
            m1 = pp.tile([128, NTILES], F32, tag="m1")
            m2 = pp.tile([128, NTILES], F32, tag="m2")
            ixm1 = pp.tile([128, NTILES], I32, tag="ixm1")
            ixs_f = pp.tile([128, NTILES], F32, tag="ixs_f")
            mnz = pp.tile([128, NTILES], F32, tag="mnz")
            xaT = pp.tile([128, NT], F32, tag="xaT")
            xbT = pp.tile([128, NT], F32, tag="xbT")
            xgb = pp.tile([128, 32 * S], F32, tag="xgb")
            hseq = pp.tile([128, NT], F32, tag="hseq")
            wgb_all = pp.tile([128, NTILES * WB], F32, tag="wgb_all")
            c_st = pp.tile([128, BS], F32, tag="c_st")
            prob_sb = pp.tile([128, NTILES], F32, tag="prob_sb")

            make_identity(nc, ident[:])
            nc.gpsimd.memset(prob_sb[:], 0.0)

            # ---- loads ----
            xt_flat = xt[:].rearrange("s b -> (s b)")
            yt_flat = yt[:].rearrange("s b -> (s b)")
            nc.sync.dma_start(
                ix_all[:], xt_flat[0:1664].rearrange("(t p) -> p t", p=128))
            nc.sync.dma_start(
                y_all[:], yt_flat[0:1664].rearrange("(t p) -> p t", p=128))
            nc.sync.dma_start(
                ixs_all[:], xt_flat[8:1672].rearrange("(t p) -> p t", p=128))
            nc.sync.dma_start(wx_sb[:], wx[:])
            nc.sync.dma_start(whh_sb[:], whh[:])
            nc.sync.dma_start(bias_sb[:], bsum[:])

            # warm the ACT sigmoid/tanh table set off the critical path
            warm = wp.tile([1, 1], F32, tag="warm")
            nc.scalar.activation(warm[:], ident[0:1, 0:1], AF.Sigmoid)

            # warm the PE HAM clock gate so the startup GEMM runs at full rate
            for _ in range(10):
                pwm = ps_mm.tile([128, 512], F32, tag="psg")
                nc.tensor.matmul(pwm[:, 0:128], ident[:], ident[:],
                                 start=True, stop=True)

            # masks: m1 = (y==0), m2 = (y==1); padding y==-1 -> 0,0
            nc.vector.tensor_copy(y_f[:], y_all[:])
            nc.vector.tensor_scalar(m1[:], y_f[:], 0.0, None, op0=OP.is_equal)
            nc.vector.tensor_scalar(m2[:], y_f[:], 1.0, None, op0=OP.is_equal)

            ex_tiles = {}

            def gather_tile(t):
                w = _tok_w(t)
                ex_t = expool.tile([128, E], F32, tag="ex")
                ex_tiles[t] = ex_t
                nc.gpsimd.indirect_dma_start(
                    out=ex_t[0:w, :], out_offset=None, in_=emb[:],
                    in_offset=bass.IndirectOffsetOnAxis(
                        ap=ix_all[0:w, t:t + 1], axis=0))

            def finish_tile(t):
                """mask + transpose gathered tile t into xaT/xbT"""
                w = _tok_w(t)
                ex_t = ex_tiles.pop(t)
                xa_t = wp.tile([128, E], F32, tag="xa")
                xb_t = wp.tile([128, E], F32, tag="xb")
                nc.vector.tensor_scalar(xa_t[0:w, :], ex_t[0:w, :],
                                        m1[0:w, t:t + 1], None, op0=OP.mult)
                nc.vector.tensor_scalar(xb_t[0:w, :], ex_t[0:w, :],
                                        m2[0:w, t:t + 1], None, op0=OP.mult)
                psa = ps_tr.tile([128, 128], F32, tag="psa")
                nc.tensor.transpose(psa[:, 0:w], xa_t[0:w, :], ident[0:w, 0:w])
                nc.vector.tensor_copy(xaT[:, 128 * t:128 * t + w], psa[:, 0:w])
                psb = ps_tr.tile([128, 128], F32, tag="psa")
                nc.tensor.transpose(psb[:, 0:w], xb_t[0:w, :], ident[0:w, 0:w])
                nc.vector.tensor_copy(xbT[:, 128 * t:128 * t + w], psb[:, 0:w])

            pending_psg = {}

            def process_tile(t):
                gather_tile(t)
                finish_tile(t)

            def gemm_a(base, w, j):
                psg = ps_mm.tile([128, 512], F32, tag="psg")
                pending_psg[(base, j)] = psg
                nc.tensor.matmul(
                    psg[:, 0:w], wx_sb[:, 128 * j:128 * (j + 1)],
                    xaT[:, base:base + w], start=True, stop=False)

            def gemm_b(base, w, j):
                psg = pending_psg.pop((base, j))
                nc.tensor.matmul(
                    psg[:, 0:w], wx_sb[:, 512 + 128 * j:512 + 128 * (j + 1)],
                    xbT[:, base:base + w], start=False, stop=True)
                dst = xgb[:, 4 * base: 4 * base + 32 * (w // 8)] \
                    .rearrange("p (q x) -> p q x", x=32)[:, :, 8 * j:8 * j + 8]
                src = psg[:, 0:w].rearrange("p (q x) -> p q x", x=8)
                nc.vector.tensor_scalar(dst, src, bias_sb[:, j:j + 1], None,
                                        op0=OP.add)

            def gemm_range(base, w, j):
                gemm_a(base, w, j)
                gemm_b(base, w, j)

            def gather_wb(t):
                w = _out_w(t)
                nc.gpsimd.indirect_dma_start(
                    out=wgb_all[0:w, WB * t:WB * (t + 1)], out_offset=None,
                    in_=wb[:],
                    in_offset=bass.IndirectOffsetOnAxis(
                        ap=ixm1[0:w, t:t + 1], axis=0))

            def out_tile(t):
                '''prob = sigmoid(h . W_out[idx] + b_out[idx]) * (X != 0)'''
                w = _out_w(t)
                pst = ps_tr.tile([128, 128], F32, tag="psa")
                nc.tensor.transpose(pst[0:w, :], hseq[:, 128 * t:128 * t + w],
                                    ident[:])
                hw_t = wp.tile([128, 128], F32, tag="hw")
                d_t = wp.tile([128, 1], F32, tag="d")
                nc.vector.tensor_tensor(out=hw_t[0:w, :], in0=pst[0:w, :],
                                        in1=wgb_all[0:w, WB * t:WB * t + H],
                                        op=OP.mult)
                nc.vector.tensor_reduce(d_t[0:w, :], hw_t[0:w, :],
                                        axis=mybir.AxisListType.X, op=OP.add)
                p_t = wp.tile([128, 1], F32, tag="p")
                nc.scalar.activation(p_t[0:w, :], d_t[0:w, :], AF.Sigmoid,
                                     bias=wgb_all[0:w, WB * t + H:WB * (t + 1)])
                nc.vector.tensor_tensor(out=prob_sb[0:w, t:t + 1],
                                        in0=p_t[0:w, :],
                                        in1=mnz[0:w, t:t + 1], op=OP.mult)

            # interleave schedule: step index -> list of thunks.
            # chunk c tokens start at step 16*_CHUNKS[c][0]; stay ahead of it.
            side = {}
            tile_steps = {1: (1, 2), 2: (11, 13), 3: (30, 34, 38, 42),
                          4: (60, 64, 68, 72)}
            gemm_steps = {1: 3, 2: 16, 3: 45, 4: 76}
            for j in range(4):                  # second half of tile 0
                side.setdefault(2 + j, []).append(("gemm0b", j))
            for c in (1, 2, 3, 4):
                t0, ntl = _CHUNKS[c]
                for k in range(ntl):
                    side.setdefault(tile_steps[c][k], []).append(
                        ("tile", t0 + k))
                for j in range(4):
                    s0 = gemm_steps[c] + 2 * j
                    side.setdefault(s0, []).append(("gemm_a", c, j))
                    side.setdefault(s0 + 1, []).append(("gemm_b", c, j))
            late_out = []
            for t in range(NTILES):             # output tiles once h is ready
                # tile t reads h(s) up to s = 16t + (w-1)//8
                smax = 16 * t + (_out_w(t) - 1) // 8
                step = max(smax + 1, 140 + 4 * t)
                if step <= S - 1:
                    side.setdefault(step, []).append(("out", t))
                else:
                    late_out.append(t)
            side.setdefault(196, []).append(("probdma",))

            # ---- chunk 0 (first 64 tokens) then the recurrence ----
            process_tile(0)
            for j in range(4):
                gemm_range(0, 64, j)

            # index prep for the W_out gathers (off the sigma(0) path)
            nc.vector.tensor_scalar(ixm1[:], ixs_all[:], 1, 0,
                                    op0=OP.subtract, op1=OP.max)
            nc.vector.tensor_copy(ixs_f[:], ixs_all[:])
            nc.vector.tensor_scalar(mnz[:], ixs_f[:], 0.0, None,
                                    op0=OP.not_equal)

            # queue every remaining gather now; the Pool engine drains them
            # in the background while the recurrence runs on PE/ACT/DVE
            for t in range(1, NTILES):
                gather_tile(t)
            for t in range(NTILES):
                gather_wb(t)

            for t in range(S):
                psr = ps_rec.tile([128, 32], F32, tag="psr")
                nc.tensor.matmul(psr[:], ident[:], xgb[:, 32 * t:32 * t + 32],
                                 start=True, stop=(t == 0))
                if t > 0:
                    hprev = hseq[:, 8 * (t - 1):8 * t]
                    for j in range(4):
                        nc.tensor.matmul(
                            psr[:, 8 * j:8 * j + 8],
                            whh_sb[:, 128 * j:128 * (j + 1)], hprev,
                            start=False, stop=(j == 3))
                # cols [i|f|o|g]; g was prescaled x2 => tanh(g) = 2*sig-1
                sig = rp.tile([128, 32], F32, tag="sig")
                nc.scalar.activation(sig[:], psr[:], AF.Sigmoid)
                # u = si*(2*sg-1) = 2*w, w = (sg-0.5)*si ; c = f*c + 2w
                w_t = rp.tile([128, 8], F32, tag="w_t")
                if t == 0:
                    nc.vector.scalar_tensor_tensor(
                        out=w_t[:], in0=sig[:, 24:32], scalar=0.5,
                        in1=sig[:, 0:8], op0=OP.subtract, op1=OP.mult)
                    nc.vector.tensor_scalar(c_st[:], w_t[:], 2.0, None,
                                            op0=OP.mult)
                else:
                    cf = rp.tile([128, 8], F32, tag="cf")
                    nc.vector.tensor_tensor(out=cf[:], in0=sig[:, 8:16],
                                            in1=c_st[:], op=OP.mult)
                    nc.vector.scalar_tensor_tensor(
                        out=w_t[:], in0=sig[:, 24:32], scalar=0.5,
                        in1=sig[:, 0:8], op0=OP.subtract, op1=OP.mult)
                    nc.vector.scalar_tensor_tensor(
                        out=c_st[:], in0=w_t[:], scalar=2.0, in1=cf[:],
                        op0=OP.mult, op1=OP.add)
                tch = rp.tile([128, 8], F32, tag="tch")
                nc.scalar.activation(tch[:], c_st[:], AF.Tanh)
                nc.vector.tensor_tensor(out=hseq[:, 8 * t:8 * t + 8],
                                        in0=sig[:, 16:24], in1=tch[:], op=OP.mult)

                for item in side.get(t, ()):
                    if item[0] == "tile":
                        finish_tile(item[1])
                    elif item[0] == "gemm0b":
                        gemm_range(64, 64, item[1])
                    elif item[0] in ("gemm_a", "gemm_b"):
                        t0, ntl = _CHUNKS[item[1]]
                        fn = gemm_a if item[0] == "gemm_a" else gemm_b
                        fn(128 * t0, min(128 * ntl, NT - 128 * t0), item[2])
                    elif item[0] == "probdma":
                        nc.sync.dma_start(
                            prob[:].rearrange("(t p) -> p t", p=128)[:, 0:12],
                            prob_sb[:, 0:12])
                    else:
                        out_tile(item[1])

            for t in late_out:
                out_tile(t)

            nc.sync.dma_start(
                prob[:].rearrange("(t p) -> p t", p=128)[:, 12:13],
                prob_sb[:, 12:13])

    nc.compile()
    return nc


_CACHED = None


def _get_kernel():
    global _CACHED
    if _CACHED is None:
        _CACHED = build_kernel()
    return _CACHED


def _prep_shared(pro_embed, W_ih, W_hh, b_ih, b_hh, W_out, b_out):
    wx_h = np.empty((128, 1024), np.float32)
    whh_h = np.empty((128, 512), np.float32)
    bias_h = np.empty((128, 4), np.float32)
    for j, g in enumerate(_GATE_SRC):
        blk = slice(g * 128, (g + 1) * 128)
        sc = 2.0 if j == 3 else 1.0   # g-gate prescale: tanh(x)=2*sig(2x)-1
        wx_h[:, j * 128:(j + 1) * 128] = sc * W_ih[blk, 0:128].T
        wx_h[:, 512 + j * 128:512 + (j + 1) * 128] = sc * W_ih[blk, 128:256].T
        whh_h[:, j * 128:(j + 1) * 128] = sc * W_hh[blk, :].T
        bias_h[:, j] = sc * (b_ih[blk] + b_hh[blk])
    wb_h = np.empty((PRO_NUM, WB), np.float32)
    wb_h[:, :H] = W_out
    wb_h[:, H] = b_out
    return dict(
        emb=np.ascontiguousarray(pro_embed, np.float32),
        wx=np.ascontiguousarray(wx_h),
        whh=np.ascontiguousarray(whh_h),
        bsum=np.ascontiguousarray(bias_h),
        wb=wb_h,
    )


def kernel(X, y, pro_embed, W_ih, W_hh, b_ih, b_hh, W_out, b_out, _trace=False,
           **_):
    X = np.asarray(X, np.int32)
    y = np.asarray(y, np.int32)
    shared = _prep_shared(np.asarray(pro_embed, np.float32),
                          np.asarray(W_ih, np.float32),
                          np.asarray(W_hh, np.float32),
                          np.asarray(b_ih, np.float32),
                          np.asarray(b_hh, np.float32),
                          np.asarray(W_out, np.float32),
                          np.asarray(b_out, np.float32))
    XT = X.T  # [200, 64]
    YT = y.T
    in_maps = []
    for c in range(N_CORES):
        cols = slice(c * BS, (c + 1) * BS)
        xtp = np.zeros((209, BS), np.int32)
        xtp[:S] = XT[:, cols]
        ytp = np.zeros((208, BS), np.int32)
        ytp[:S] = YT[:, cols]
        in_maps.append(dict(xt=xtp, yt=ytp, **shared))

    nc = _get_kernel()
    res = run_bass_kernel_spmd(nc, in_maps, core_ids=list(range(N_CORES)),
                               trace=_trace)
    out = np.empty((B, S - 1), np.float32)
    for c in range(N_CORES):
        flat = res.results[c]["prob"][:NOUT].reshape(S - 1, BS)
        out[c * BS:(c + 1) * BS, :] = flat.T
    if _trace:
        return out, res
    return out


# revision 24
# speedup vs baseline: 263.1610x; 1.0062x over previous
"""DKT-PEBG kernel for Trainium2 (8 NeuronCores, batch-parallel).

Model: embedding lookup -> masked concat -> LSTM(128) -> per-token output
probability via gathered W_out rows (avoids materializing [B,S,10000]).

Sharding: data-parallel over batch. Core c handles batch rows [8c, 8c+8).
No collectives; host splits inputs / concatenates outputs.

Shapes (hardcoded): B=64, S=200, E=H=128, PRO_NUM=10000.

Recurrence trick: gate order [i,f,o,g] with the g-gate pre-activation
prescaled by 2 on the host, so one Sigmoid over all 4 gates suffices:
tanh(g) = 2*sigmoid(2g) - 1. Input-GEMM chunks and gathers are emitted
interleaved with the first recurrence steps so the scheduler pipelines
them instead of serializing ~45us of startup.
"""

import numpy as np

import concourse.bass as bass
import concourse.bacc as bacc
import concourse.mybir as mybir
import concourse.tile as tile
from concourse.bass_utils import run_bass_kernel_spmd
from concourse.masks import make_identity

B, S = 64, 200
E = 128
H = 128
PRO_NUM = 10000
N_CORES = 8
BS = B // N_CORES              # 8 batch rows per core
NT = BS * S                    # 1600 tokens per core, token n = 8*s + b
NTILES = 13                    # ceil(1600/128); tile 12 has 64 valid tokens
NOUT = BS * (S - 1)            # 1592 output tokens
WB = H + 1                     # gathered W_out row + bias
F32 = mybir.dt.float32
I32 = mybir.dt.int32

_GATE_SRC = (0, 1, 3, 2)       # col blocks [i, f, o, g] <- W_ih row blocks (i,f,g,o)

# input-GEMM chunks in tiles: (first_tile, n_tiles); chunk 0 small so the
# recurrence can start early
_CHUNKS = ((0, 1), (1, 2), (3, 2), (5, 4), (9, 4))


def _tok_w(t):
    return 128 if t < NTILES - 1 else NT - 128 * (NTILES - 1)


def _out_w(t):
    return 128 if t < NTILES - 1 else NOUT - 128 * (NTILES - 1)


def build_kernel():
    nc = bacc.Bacc("TRN2", target_bir_lowering=False, debug=False,
                   num_devices=N_CORES)

    # ---- I/O ----
    xt = nc.dram_tensor("xt", [209, BS], I32, kind="ExternalInput")   # X.T slice, padded
    yt = nc.dram_tensor("yt", [208, BS], I32, kind="ExternalInput")   # y.T slice, padded
    emb = nc.dram_tensor("emb", [PRO_NUM, E], F32, kind="ExternalInput")
    wx = nc.dram_tensor("wx", [128, 1024], F32, kind="ExternalInput")  # W_ih.T blocks [A|B]
    whh = nc.dram_tensor("whh", [128, 512], F32, kind="ExternalInput")  # W_hh.T blocks
    bsum = nc.dram_tensor("bsum", [128, 4], F32, kind="ExternalInput")  # b_ih+b_hh blocks
    wb = nc.dram_tensor("wb", [PRO_NUM, WB], F32, kind="ExternalInput")  # [W_out | b_out]
    prob = nc.dram_tensor("prob", [NTILES * 128], F32, kind="ExternalOutput")

    AF = mybir.ActivationFunctionType
    OP = mybir.AluOpType

    with tile.TileContext(nc) as tc:
        with (
            tc.tile_pool(name="persist", bufs=1) as pp,
            tc.tile_pool(name="work", bufs=4) as wp,
            tc.tile_pool(name="exp", bufs=13) as expool,
            tc.tile_pool(name="rec", bufs=4) as rp,
            tc.tile_pool(name="ps_tr", bufs=2, space="PSUM") as ps_tr,
            tc.tile_pool(name="ps_mm", bufs=3, space="PSUM") as ps_mm,
            tc.tile_pool(name="ps_rec", bufs=3, space="PSUM") as ps_rec,
        ):
            # ---- persistent SBUF ----
            ident = pp.tile([128, 128], F32, tag="ident")
            wx_sb = pp.tile([128, 1024], F32, tag="wx_sb")
            whh_sb = pp.tile([128, 512], F32, tag="whh_sb")
            bias_sb = pp.tile([128, 4], F32, tag="bias_sb")
            ix_all = pp.tile([128, NTILES], I32, tag="ix_all")
            ixs_all = pp.tile([128, NTILES], I32, tag="ixs_all")
            y_all = pp.tile([128, NTILES], I32, tag="y_all")
            y_f = pp.tile([128, NTILES], F32, tag="y_f")
            m1 = pp.tile([128, NTILES], F32, tag="m1")
            m2 = pp.tile([128, NTILES], F32, tag="m2")
            ixm1 = pp.tile([128, NTILES], I32, tag="ixm1")
            ixs_f = pp.tile([128, NTILES], F32, tag="ixs_f")
            mnz = pp.tile([128, NTILES], F32, tag="mnz")
            xaT = pp.tile([128, NT], F32, tag="xaT")
            xbT = pp.tile([128, NT], F32, tag="xbT")
            xgb = pp.tile([128, 32 * S], F32, tag="xgb")
            hseq = pp.tile([128, NT], F32, tag="hseq")
            wgb_all = pp.tile([128, NTILES * WB], F32, tag="wgb_all")
            c_st = pp.tile([128, BS], F32, tag="c_st")
            prob_sb = pp.tile([128, NTILES], F32, tag="prob_sb")

            make_identity(nc, ident[:])
            nc.gpsimd.memset(prob_sb[:], 0.0)

            # ---- loads ----
            xt_flat = xt[:].rearrange("s b -> (s b)")
            yt_flat = yt[:].rearrange("s b -> (s b)")
            nc.sync.dma_start(
                ix_all[:], xt_flat[0:1664].rearrange("(t p) -> p t", p=128))
            nc.sync.dma_start(
                y_all[:], yt_flat[0:1664].rearrange("(t p) -> p t", p=128))
            nc.sync.dma_start(
                ixs_all[:], xt_flat[8:1672].rearrange("(t p) -> p t", p=128))
            nc.sync.dma_start(wx_sb[:], wx[:])
            nc.sync.dma_start(whh_sb[:], whh[:])
            nc.sync.dma_start(bias_sb[:], bsum[:])

            # warm the ACT sigmoid/tanh table set off the critical path
            warm = wp.tile([1, 1], F32, tag="warm")
            nc.scalar.activation(warm[:], ident[0:1, 0:1], AF.Sigmoid)

            # warm the PE HAM clock gate so the startup GEMM runs at full rate
            for _ in range(10):
                pwm = ps_mm.tile([128, 512], F32, tag="psg")
                nc.tensor.matmul(pwm[:, 0:128], ident[:], ident[:],
                                 start=True, stop=True)

            # masks: m1 = (y==0), m2 = (y==1); padding y==-1 -> 0,0
            nc.vector.tensor_copy(y_f[:], y_all[:])
            nc.vector.tensor_scalar(m1[:], y_f[:], 0.0, None, op0=OP.is_equal)
            nc.vector.tensor_scalar(m2[:], y_f[:], 1.0, None, op0=OP.is_equal)

            ex_tiles = {}

            def gather_tile(t):
                w = _tok_w(t)
                ex_t = expool.tile([128, E], F32, tag="ex")
                ex_tiles[t] = ex_t
                nc.gpsimd.indirect_dma_start(
                    out=ex_t[0:w, :], out_offset=None, in_=emb[:],
                    in_offset=bass.IndirectOffsetOnAxis(
                        ap=ix_all[0:w, t:t + 1], axis=0))

            def finish_tile(t):
                """mask + transpose gathered tile t into xaT/xbT"""
                w = _tok_w(t)
                ex_t = ex_tiles.pop(t)
                xa_t = wp.tile([128, E], F32, tag="xa")
                xb_t = wp.tile([128, E], F32, tag="xb")
                nc.vector.tensor_scalar(xa_t[0:w, :], ex_t[0:w, :],
                                        m1[0:w, t:t + 1], None, op0=OP.mult)
                nc.vector.tensor_scalar(xb_t[0:w, :], ex_t[0:w, :],
                                        m2[0:w, t:t + 1], None, op0=OP.mult)
                psa = ps_tr.tile([128, 128], F32, tag="psa")
                nc.tensor.transpose(psa[:, 0:w], xa_t[0:w, :], ident[0:w, 0:w])
                nc.vector.tensor_copy(xaT[:, 128 * t:128 * t + w], psa[:, 0:w])
                psb = ps_tr.tile([128, 128], F32, tag="psa")
                nc.tensor.transpose(psb[:, 0:w], xb_t[0:w, :], ident[0:w, 0:w])
                nc.vector.tensor_copy(xbT[:, 128 * t:128 * t + w], psb[:, 0:w])

            pending_psg = {}

            def process_tile(t):
                gather_tile(t)
                finish_tile(t)

            def gemm_a(base, w, j):
                psg = ps_mm.tile([128, 512], F32, tag="psg")
                pending_psg[(base, j)] = psg
                nc.tensor.matmul(
                    psg[:, 0:w], wx_sb[:, 128 * j:128 * (j + 1)],
                    xaT[:, base:base + w], start=True, stop=False)

            def gemm_b(base, w, j):
                psg = pending_psg.pop((base, j))
                nc.tensor.matmul(
                    psg[:, 0:w], wx_sb[:, 512 + 128 * j:512 + 128 * (j + 1)],
                    xbT[:, base:base + w], start=False, stop=True)
                dst = xgb[:, 4 * base: 4 * base + 32 * (w // 8)] \
                    .rearrange("p (q x) -> p q x", x=32)[:, :, 8 * j:8 * j + 8]
                src = psg[:, 0:w].rearrange("p (q x) -> p q x", x=8)
                nc.vector.tensor_scalar(dst, src, bias_sb[:, j:j + 1], None,
                                        op0=OP.add)

            def gemm_range(base, w, j):
                gemm_a(base, w, j)
                gemm_b(base, w, j)

            def gather_wb(t):
                w = _out_w(t)
                nc.gpsimd.indirect_dma_start(
                    out=wgb_all[0:w, WB * t:WB * (t + 1)], out_offset=None,
                    in_=wb[:],
                    in_offset=bass.IndirectOffsetOnAxis(
                        ap=ixm1[0:w, t:t + 1], axis=0))

            def out_tile(t):
                '''prob = sigmoid(h . W_out[idx] + b_out[idx]) * (X != 0)'''
                w = _out_w(t)
                pst = ps_tr.tile([128, 128], F32, tag="psa")
                nc.tensor.transpose(pst[0:w, :], hseq[:, 128 * t:128 * t + w],
                                    ident[:])
                hw_t = wp.tile([128, 128], F32, tag="hw")
                d_t = wp.tile([128, 1], F32, tag="d")
                nc.vector.scalar_tensor_tensor(
                    out=hw_t[0:w, :], in0=pst[0:w, :], scalar=1.0,
                    in1=wgb_all[0:w, WB * t:WB * t + H],
                    op0=OP.mult, op1=OP.mult, accum_out=d_t[0:w, :])
                p_t = wp.tile([128, 1], F32, tag="p")
                nc.scalar.activation(p_t[0:w, :], d_t[0:w, :], AF.Sigmoid,
                                     bias=wgb_all[0:w, WB * t + H:WB * (t + 1)])
                nc.vector.tensor_tensor(out=prob_sb[0:w, t:t + 1],
                                        in0=p_t[0:w, :],
                                        in1=mnz[0:w, t:t + 1], op=OP.mult)

            # interleave schedule: step index -> list of thunks.
            # chunk c tokens start at step 16*_CHUNKS[c][0]; stay ahead of it.
            side = {}
            tile_steps = {1: (1, 2), 2: (11, 13), 3: (30, 34, 38, 42),
                          4: (60, 64, 68, 72)}
            gemm_steps = {1: 3, 2: 16, 3: 45, 4: 76}
            for j in range(4):                  # second half of tile 0
                side.setdefault(2 + j, []).append(("gemm0b", j))
            for c in (1, 2, 3, 4):
                t0, ntl = _CHUNKS[c]
                for k in range(ntl):
                    side.setdefault(tile_steps[c][k], []).append(
                        ("tile", t0 + k))
                for j in range(4):
                    s0 = gemm_steps[c] + 2 * j
                    side.setdefault(s0, []).append(("gemm_a", c, j))
                    side.setdefault(s0 + 1, []).append(("gemm_b", c, j))
            late_out = []
            for t in range(NTILES):             # output tiles once h is ready
                # tile t reads h(s) up to s = 16t + (w-1)//8
                smax = 16 * t + (_out_w(t) - 1) // 8
                step = max(smax + 1, 140 + 4 * t)
                if step <= S - 1:
                    side.setdefault(step, []).append(("out", t))
                else:
                    late_out.append(t)
            side.setdefault(196, []).append(("probdma",))

            # ---- chunk 0 (first 64 tokens) then the recurrence ----
            process_tile(0)
            for j in range(4):
                gemm_range(0, 64, j)

            # index prep for the W_out gathers (off the sigma(0) path)
            nc.vector.tensor_scalar(ixm1[:], ixs_all[:], 1, 0,
                                    op0=OP.subtract, op1=OP.max)
            nc.vector.tensor_copy(ixs_f[:], ixs_all[:])
            nc.vector.tensor_scalar(mnz[:], ixs_f[:], 0.0, None,
                                    op0=OP.not_equal)

            # queue every remaining gather now; the Pool engine drains them
            # in the background while the recurrence runs on PE/ACT/DVE
            for t in range(1, NTILES):
                gather_tile(t)
            for t in range(NTILES):
                gather_wb(t)

            for t in range(S):
                psr = ps_rec.tile([128, 32], F32, tag="psr")
                nc.tensor.matmul(psr[:], ident[:], xgb[:, 32 * t:32 * t + 32],
                                 start=True, stop=(t == 0))
                if t > 0:
                    hprev = hseq[:, 8 * (t - 1):8 * t]
                    for j in range(4):
                        nc.tensor.matmul(
                            psr[:, 8 * j:8 * j + 8],
                            whh_sb[:, 128 * j:128 * (j + 1)], hprev,
                            start=False, stop=(j == 3))
                # cols [i|f|o|g]; g was prescaled x2 => tanh(g) = 2*sig-1
                sig = rp.tile([128, 32], F32, tag="sig")
                nc.scalar.activation(sig[:], psr[:], AF.Sigmoid)
                # u = si*(2*sg-1) = 2*w, w = (sg-0.5)*si ; c = f*c + 2w
                w_t = rp.tile([128, 8], F32, tag="w_t")
                if t == 0:
                    nc.vector.scalar_tensor_tensor(
                        out=w_t[:], in0=sig[:, 24:32], scalar=0.5,
                        in1=sig[:, 0:8], op0=OP.subtract, op1=OP.mult)
                    nc.vector.tensor_scalar(c_st[:], w_t[:], 2.0, None,
                                            op0=OP.mult)
                else:
                    cf = rp.tile([128, 8], F32, tag="cf")
                    nc.vector.tensor_tensor(out=cf[:], in0=sig[:, 8:16],
                                            in1=c_st[:], op=OP.mult)
                    nc.vector.scalar_tensor_tensor(
                        out=w_t[:], in0=sig[:, 24:32], scalar=0.5,
                        in1=sig[:, 0:8], op0=OP.subtract, op1=OP.mult)
                    nc.vector.scalar_tensor_tensor(
                        out=c_st[:], in0=w_t[:], scalar=2.0, in1=cf[:],
                        op0=OP.mult, op1=OP.add)
                tch = rp.tile([128, 8], F32, tag="tch")
                nc.scalar.activation(tch[:], c_st[:], AF.Tanh)
                nc.vector.tensor_tensor(out=hseq[:, 8 * t:8 * t + 8],
                                        in0=sig[:, 16:24], in1=tch[:], op=OP.mult)

                for item in side.get(t, ()):
                    if item[0] == "tile":
                        finish_tile(item[1])
                    elif item[0] == "gemm0b":
                        gemm_range(64, 64, item[1])
                    elif item[0] in ("gemm_a", "gemm_b"):
                        t0, ntl = _CHUNKS[item[1]]
                        fn = gemm_a if item[0] == "gemm_a" else gemm_b
                        fn(128 * t0, min(128 * ntl, NT - 128 * t0), item[2])
                    elif item[0] == "probdma":
                        nc.sync.dma_start(
                            prob[:].rearrange("(t p) -> p t", p=128)[:, 0:12],
                            prob_sb[:, 0:12])
                    else:
                        out_tile(item[1])

            for t in late_out:
                out_tile(t)

            nc.sync.dma_start(
                prob[:].rearrange("(t p) -> p t", p=128)[:, 12:13],
                prob_sb[:, 12:13])

    nc.compile()
    return nc


_CACHED = None


def _get_kernel():
    global _CACHED
    if _CACHED is None:
        _CACHED = build_kernel()
    return _CACHED


def _prep_shared(pro_embed, W_ih, W_hh, b_ih, b_hh, W_out, b_out):
    wx_h = np.empty((128, 1024), np.float32)
    whh_h = np.empty((128, 512), np.float32)
    bias_h = np.empty((128, 4), np.float32)
    for j, g in enumerate(_GATE_SRC):
        blk = slice(g * 128, (g + 1) * 128)
        sc = 2.0 if j == 3 else 1.0   # g-gate prescale: tanh(x)=2*sig(2x)-1
        wx_h[:, j * 128:(j + 1) * 128] = sc * W_ih[blk, 0:128].T
        wx_h[:, 512 + j * 128:512 + (j + 1) * 128] = sc * W_ih[blk, 128:256].T
        whh_h[:, j * 128:(j + 1) * 128] = sc * W_hh[blk, :].T
        bias_h[:, j] = sc * (b_ih[blk] + b_hh[blk])
    wb_h = np.empty((PRO_NUM, WB), np.float32)
    wb_h[:, :H] = W_out
    wb_h[:, H] = b_out
    return dict(
        emb=np.ascontiguousarray(pro_embed, np.float32),
        wx=np.ascontiguousarray(wx_h),
        whh=np.ascontiguousarray(whh_h),
        bsum=np.ascontiguousarray(bias_h),
        wb=wb_h,
    )


def kernel(X, y, pro_embed, W_ih, W_hh, b_ih, b_hh, W_out, b_out, _trace=False,
           **_):
    X = np.asarray(X, np.int32)
    y = np.asarray(y, np.int32)
    shared = _prep_shared(np.asarray(pro_embed, np.float32),
                          np.asarray(W_ih, np.float32),
                          np.asarray(W_hh, np.float32),
                          np.asarray(b_ih, np.float32),
                          np.asarray(b_hh, np.float32),
                          np.asarray(W_out, np.float32),
                          np.asarray(b_out, np.float32))
    XT = X.T  # [200, 64]
    YT = y.T
    in_maps = []
    for c in range(N_CORES):
        cols = slice(c * BS, (c + 1) * BS)
        xtp = np.zeros((209, BS), np.int32)
        xtp[:S] = XT[:, cols]
        ytp = np.zeros((208, BS), np.int32)
        ytp[:S] = YT[:, cols]
        in_maps.append(dict(xt=xtp, yt=ytp, **shared))

    nc = _get_kernel()
    res = run_bass_kernel_spmd(nc, in_maps, core_ids=list(range(N_CORES)),
                               trace=_trace)
    out = np.empty((B, S - 1), np.float32)
    for c in range(N_CORES):
        flat = res.results[c]["prob"][:NOUT].reshape(S - 1, BS)
        out[c * BS:(c + 1) * BS, :] = flat.T
    if _trace:
        return out, res
    return out
